# revision 5
# baseline (speedup 1.0000x reference)
"""GNN NodeModel kernel for 8 Trainium2 NeuronCores (Bass/Tile).

Pipeline (per the reference nn.Module):
  scatter_max / scatter_mean / scatter_add of edge_attr by edge dest ->
  h = [x, u[batch], smax, smean, ssum]  (N x 832) ->
  Linear(832->1024) -> BatchNorm(train stats) -> ReLU ->
  Linear(1024->1024) -> BatchNorm(train stats)  => [N, 1024]

Sharding: nodes split into 8 contiguous shards of 6250; each core gets its
shard's incoming edges (bucketed by col on host).  Within a shard nodes are
degree-sorted and packed into 13 tiles of 512 (last 106 valid + padding).
Edges are laid out host-side in a padded ELL format so the device scatter is
a dense max/add accumulation.  All GEMMs run transposed (channels on
partitions, nodes on the free dim) in bf16 with fp32 PSUM accumulate; BN
statistics are computed per-channel with bn_stats and all-reduced across the
8 cores on-device.  BN biases b1/b2 cancel inside train-mode BatchNorm and
are not used.
"""

import numpy as np
import ml_dtypes

import concourse.bass as bass
import concourse.bacc as bacc
import concourse.tile as tile
from concourse import mybir
from concourse.bass_utils import run_bass_kernel_spmd

BF16 = mybir.dt.bfloat16
F32 = mybir.dt.float32

NCORES = 8
N = 50000
E = 800000
XI = 512
EI = 64
UI = 128
HS = 1024
G = 8
EPS = 1e-5
CIN = XI + 3 * EI + UI  # 832

NSH = N // NCORES          # 6250 nodes per core
TW = 512                   # node-tile width (free dim)
NT = 13                    # tiles per core (12*512 + 106)
NCOL = NT * TW             # 6656 padded columns
LASTW = NSH - (NT - 1) * TW  # 106
KT1 = 7                    # GEMM1 k-tiles (896 = 832 + 64 pad)
KT2 = 8                    # GEMM2 k-tiles
MT = HS // 128             # 8 channel tiles
NEG = -1000.0              # ELL pad for the max reduction


# ----------------------------------------------------------------------------
# Host-side sharding / layout prep
# ----------------------------------------------------------------------------

def _host_prep(x, edge_attr, u, w1, w2, g1, be1, g2, be2, edge_index, batch):
    bf = ml_dtypes.bfloat16
    col = np.asarray(edge_index[1])
    deg_all = np.bincount(col, minlength=N).astype(np.int64)

    shard_of_edge = col // NSH

    # per-core degree-sorted node order and per-tile slot counts
    perms = []
    degs_sorted = []
    for c in range(NCORES):
        dc = deg_all[c * NSH:(c + 1) * NSH]
        perm = np.argsort(-dc, kind="stable")
        perms.append(perm)
        degs_sorted.append(dc[perm])

    # global per-tile slot counts (same on every core so one NEFF fits all),
    # padded to a multiple of 4 for the pair-tree reduction
    D = []
    for t in range(NT):
        m = 0
        for c in range(NCORES):
            seg = degs_sorted[c][t * TW:(t + 1) * TW]
            if seg.size:
                m = max(m, int(seg.max()))
        D.append(-(-m // 4) * 4)
    offs = np.concatenate([[0], np.cumsum(D)]).astype(np.int64)
    S = int(offs[-1])

    per_core = []
    ea_bf = np.asarray(edge_attr, np.float32).astype(bf)
    x_f = np.asarray(x, np.float32)
    batch_np = np.asarray(batch)

    for c in range(NCORES):
        perm = perms[c]
        inv = np.empty(NSH, np.int64)
        inv[perm] = np.arange(NSH)

        emask = shard_of_edge == c
        l_orig = col[emask] - c * NSH          # local node id
        l = inv[l_orig]                         # degree-sorted local id
        vals = ea_bf[emask]                     # [Ec, 64] bf16

        order = np.argsort(l, kind="stable")
        l_s = l[order]
        vals_s = vals[order]
        first = np.searchsorted(l_s, l_s, side="left")
        slot = np.arange(l_s.size) - first      # rank within node

        t_arr = l_s // TW
        rem = l_s % TW
        g_arr = rem // 256
        j_arr = rem % 256
        s_glob = offs[t_arr] + slot

        ell_max = np.full((2, 64, S, 256), NEG, dtype=bf)
        ell_sum = np.zeros((2, 64, S, 256), dtype=bf)
        ell_max[g_arr, :, s_glob, j_arr] = vals_s
        ell_sum[g_arr, :, s_glob, j_arr] = vals_s

        # x^T [512, NCOL], permuted + zero-padded
        xT = np.zeros((XI, NCOL), dtype=bf)
        xT[:, :NSH] = x_f[c * NSH:(c + 1) * NSH][perm].T.astype(bf)

        # u one-hot [8, NCOL]
        onehot = np.zeros((G, NCOL), dtype=bf)
        bvals = batch_np[c * NSH:(c + 1) * NSH][perm]
        onehot[bvals, np.arange(NSH)] = bf(1.0)

        # degrees in stacked-group layout [2, NT*256]
        deg2 = np.zeros((2, NT * 256), dtype=bf)
        dsort = degs_sorted[c].astype(np.float32)
        dpad = np.zeros(NCOL, np.float32)
        dpad[:NSH] = dsort
        dv = dpad.reshape(NT, 2, 256)
        deg2[0] = dv[:, 0, :].reshape(-1)
        deg2[1] = dv[:, 1, :].reshape(-1)

        per_core.append(dict(
            xT=np.ascontiguousarray(xT),
            ellmax=np.ascontiguousarray(ell_max.reshape(128, S * 256)),
            ellsum=np.ascontiguousarray(ell_sum.reshape(128, S * 256)),
            onehot=np.ascontiguousarray(onehot),
            deg2=np.ascontiguousarray(deg2),
        ))

    # replicated weights
    w1 = np.asarray(w1, np.float32)
    w2 = np.asarray(w2, np.float32)
    w1T = np.zeros((KT1 * 128, HS), dtype=bf)
    w1T[0:512] = w1[:, 0:512].T.astype(bf)        # x block
    w1T[512:640] = w1[:, 512:640].T.astype(bf)    # u block
    w1T[640:704] = w1[:, 640:704].T.astype(bf)    # smax  (k5 top)
    w1T[704:768] = w1[:, 768:832].T.astype(bf)    # ssum  (k5 bottom)
    w1T[768:832] = w1[:, 704:768].T.astype(bf)    # smean (k6 top)
    w2T = np.ascontiguousarray(w2.T.astype(bf))

    def cvec(v):
        return np.ascontiguousarray(
            np.asarray(v, np.float32).reshape(MT, 128).T)

    shared = dict(
        w1T=np.ascontiguousarray(w1T),
        w2T=w2T,
        u8=np.asarray(u, np.float32).astype(bf),
        g1t=cvec(g1), be1t=cvec(be1), g2t=cvec(g2), be2t=cvec(be2),
    )
    return per_core, shared, perms, D, S


# ----------------------------------------------------------------------------
# Device kernel
# ----------------------------------------------------------------------------

def _build(D, S):
    nc = bacc.Bacc("TRN2", target_bir_lowering=False, debug=False,
                   num_devices=NCORES)

    t_xT = nc.dram_tensor("xT", [XI, NCOL], BF16, kind="ExternalInput")
    t_emax = nc.dram_tensor("ellmax", [128, S * 256], BF16, kind="ExternalInput")
    t_esum = nc.dram_tensor("ellsum", [128, S * 256], BF16, kind="ExternalInput")
    t_oneh = nc.dram_tensor("onehot", [G, NCOL], BF16, kind="ExternalInput")
    t_deg2 = nc.dram_tensor("deg2", [2, NT * 256], BF16, kind="ExternalInput")
    t_u8 = nc.dram_tensor("u8", [G, UI], BF16, kind="ExternalInput")
    t_w1T = nc.dram_tensor("w1T", [KT1 * 128, HS], BF16, kind="ExternalInput")
    t_w2T = nc.dram_tensor("w2T", [HS, HS], BF16, kind="ExternalInput")
    t_g1 = nc.dram_tensor("g1t", [128, MT], F32, kind="ExternalInput")
    t_be1 = nc.dram_tensor("be1t", [128, MT], F32, kind="ExternalInput")
    t_g2 = nc.dram_tensor("g2t", [128, MT], F32, kind="ExternalInput")
    t_be2 = nc.dram_tensor("be2t", [128, MT], F32, kind="ExternalInput")
    t_out = nc.dram_tensor("outT", [HS, NCOL], F32, kind="ExternalOutput")

    offs = np.concatenate([[0], np.cumsum(D)]).astype(np.int64)
    AMAX = mybir.AluOpType.max
    AADD = mybir.AluOpType.add
    ACopy = mybir.ActivationFunctionType.Copy
    AIdent = mybir.ActivationFunctionType.Identity
    ARelu = mybir.ActivationFunctionType.Relu
    ASqrt = mybir.ActivationFunctionType.Sqrt

    with tile.TileContext(nc) as tc:
        with (
            tc.tile_pool(name="wp", bufs=1) as wp,
            tc.tile_pool(name="y1p", bufs=1) as y1p,
            tc.tile_pool(name="hp", bufs=2) as hp,
            tc.tile_pool(name="ellp", bufs=2) as ellp,
            tc.tile_pool(name="accp", bufs=2) as accp,
            tc.tile_pool(name="smallp", bufs=2) as smallp,
            tc.tile_pool(name="evp", bufs=2) as evp,
            tc.tile_pool(name="statp", bufs=1) as statp,
            tc.tile_pool(name="psg", bufs=3, space="PSUM") as psg,
            tc.tile_pool(name="psu", bufs=2, space="PSUM") as psu,
            tc.tile_pool(name="dramp", bufs=1, space="DRAM") as dramp,
        ):
            # ---- resident constants ----
            w1t = []
            for k in range(KT1):
                wt_ = wp.tile([128, HS], BF16, tag=f"w1_{k}")
                nc.sync.dma_start(out=wt_[:], in_=t_w1T[k * 128:(k + 1) * 128, :])
                w1t.append(wt_)
            w2t = []
            for k in range(KT2):
                wt_ = wp.tile([128, HS], BF16, tag=f"w2_{k}")
                nc.sync.dma_start(out=wt_[:], in_=t_w2T[k * 128:(k + 1) * 128, :])
                w2t.append(wt_)
            u_sb = wp.tile([G, UI], BF16, tag="u8")
            nc.sync.dma_start(out=u_sb[:], in_=t_u8[:])
            g1_sb = wp.tile([128, MT], F32, tag="g1")
            be1_sb = wp.tile([128, MT], F32, tag="be1")
            g2_sb = wp.tile([128, MT], F32, tag="g2")
            be2_sb = wp.tile([128, MT], F32, tag="be2")
            for tt, sb in ((t_g1, g1_sb), (t_be1, be1_sb),
                           (t_g2, g2_sb), (t_be2, be2_sb)):
                nc.sync.dma_start(out=sb[:], in_=tt[:])
            # selection matrix for broadcasting [2, x] rows to partition halves
            sel2_np = np.zeros((2, 128), dtype=ml_dtypes.bfloat16)
            sel2_np[0, 0:64] = 1.0
            sel2_np[1, 64:128] = 1.0
            sel2_dram = nc.inline_tensor(sel2_np, name="sel2c")
            sel2 = wp.tile([2, 128], BF16, tag="sel2")
            nc.sync.dma_start(out=sel2[:], in_=sel2_dram[:])

            y1 = [[y1p.tile([128, TW], BF16, tag=f"y1_{m}_{t}",
                            name=f"y1_{m}_{t}")
                   for t in range(NT)] for m in range(MT)]
            st1 = [statp.tile([128, NT, 6], F32, tag=f"st1_{m}", name=f"st1_{m}")
                   for m in range(MT)]
            st2 = [statp.tile([128, NT, 6], F32, tag=f"st2_{m}", name=f"st2_{m}")
                   for m in range(MT)]

            # ---------------- phase 1: scatter + GEMM1 + stats1 ----------------
            for t in range(NT):
                wvalid = TW if t < NT - 1 else LASTW
                h_t = hp.tile([128, KT1, TW], BF16, tag="h")
                for k in range(4):
                    nc.sync.dma_start(
                        out=h_t[:, k, :],
                        in_=t_xT[k * 128:(k + 1) * 128, t * TW:(t + 1) * TW])

                # u[batch] via one-hot matmul
                oh_t = smallp.tile([G, TW], BF16, tag="oh")
                nc.sync.dma_start(out=oh_t[:], in_=t_oneh[:, t * TW:(t + 1) * TW])
                ps_u = psu.tile([128, TW], F32, space="PSUM", tag="psu")
                nc.tensor.matmul(out=ps_u[:], lhsT=u_sb[:], rhs=oh_t[:],
                                 start=True, stop=True)
                nc.scalar.activation(out=h_t[:, 4, :], in_=ps_u[:], func=ACopy)

                # mask / inverse-count, broadcast to partition halves
                dg_t = smallp.tile([2, 256], BF16, tag="dg")
                nc.sync.dma_start(out=dg_t[:], in_=t_deg2[:, t * 256:(t + 1) * 256])
                msk = smallp.tile([2, 256], BF16, tag="msk")
                nc.vector.tensor_scalar_min(msk[:], dg_t[:], 1.0)
                dgc = smallp.tile([2, 256], F32, tag="dgc")
                nc.vector.tensor_scalar_max(dgc[:], dg_t[:], 1.0)
                invf = smallp.tile([2, 256], F32, tag="invf")
                nc.vector.reciprocal(out=invf[:], in_=dgc[:])
                invd = smallp.tile([2, 256], BF16, tag="invd")
                nc.vector.tensor_copy(out=invd[:], in_=invf[:])
                ps_b = psu.tile([128, TW], F32, space="PSUM", tag="psb")
                nc.tensor.matmul(out=ps_b[:, 0:256], lhsT=sel2[:], rhs=invd[:],
                                 start=True, stop=True)
                nc.tensor.matmul(out=ps_b[:, 256:512], lhsT=sel2[:], rhs=msk[:],
                                 start=True, stop=True)
                invmask = evp.tile([128, TW], BF16, tag="invmask")
                nc.scalar.activation(out=invmask[:], in_=ps_b[:], func=ACopy)

                # ELL scatter: pair-tree max / sum over D[t] slots
                ngr = D[t] // 4
                a2m = accp.tile([128, 2, 256], BF16, tag="a2m")
                a2s = accp.tile([128, 2, 256], BF16, tag="a2s")
                for gi in range(ngr):
                    base = (offs[t] + 4 * gi) * 256
                    cm = ellp.tile([128, 4, 256], BF16, tag="cm")
                    cs = ellp.tile([128, 4, 256], BF16, tag="cs")
                    nc.sync.dma_start(out=cm[:], in_=t_emax[:, base:base + 1024])
                    nc.sync.dma_start(out=cs[:], in_=t_esum[:, base:base + 1024])
                    if gi == 0:
                        nc.vector.tensor_tensor(out=a2m[:], in0=cm[:, 0:2, :],
                                                in1=cm[:, 2:4, :], op=AMAX)
                        nc.vector.tensor_tensor(out=a2s[:], in0=cs[:, 0:2, :],
                                                in1=cs[:, 2:4, :], op=AADD)
                    else:
                        nc.vector.tensor_tensor(out=a2m[:], in0=a2m[:],
                                                in1=cm[:, 0:2, :], op=AMAX)
                        nc.vector.tensor_tensor(out=a2m[:], in0=a2m[:],
                                                in1=cm[:, 2:4, :], op=AMAX)
                        nc.vector.tensor_tensor(out=a2s[:], in0=a2s[:],
                                                in1=cs[:, 0:2, :], op=AADD)
                        nc.vector.tensor_tensor(out=a2s[:], in0=a2s[:],
                                                in1=cs[:, 2:4, :], op=AADD)

                accm = accp.tile([128, 256], BF16, tag="accm")
                accs = accp.tile([128, 256], BF16, tag="accs")
                if ngr > 0:
                    nc.vector.tensor_tensor(out=accm[:], in0=a2m[:, 0, :],
                                            in1=a2m[:, 1, :], op=AMAX)
                    nc.vector.tensor_tensor(out=accs[:], in0=a2s[:, 0, :],
                                            in1=a2s[:, 1, :], op=AADD)
                else:
                    nc.gpsimd.memset(accm[:], 0.0)
                    nc.gpsimd.memset(accs[:], 0.0)

                # mask empty nodes; smean = ssum * inv
                nc.vector.tensor_mul(out=accm[:], in0=accm[:],
                                     in1=invmask[:, 256:512])
                smean = accp.tile([128, 256], BF16, tag="smean")
                nc.vector.tensor_mul(out=smean[:], in0=accs[:],
                                     in1=invmask[:, 0:256])

                # restack [2x64-feat groups, 256] -> [64-feat, 512] rows of h
                # k5 = [smax ; ssum], k6 = [smean ; 0]
                nc.vector.tensor_copy(out=h_t[0:64, 5, 0:256], in_=accm[0:64, :])
                nc.sync.dma_start(out=h_t[0:64, 5, 256:512], in_=accm[64:128, :])
                nc.sync.dma_start(out=h_t[64:128, 5, 0:256], in_=accs[0:64, :])
                nc.vector.tensor_copy(out=h_t[64:128, 5, 256:512], in_=accs[64:128, :])
                nc.vector.tensor_copy(out=h_t[0:64, 6, 0:256], in_=smean[0:64, :])
                nc.sync.dma_start(out=h_t[0:64, 6, 256:512], in_=smean[64:128, :])
                nc.gpsimd.memset(h_t[64:128, 6, :], 0.0)

                # GEMM1 + stats + evac
                for m in range(MT):
                    ps = psg.tile([128, TW], F32, space="PSUM", tag="psg")
                    for k in range(KT1):
                        nc.tensor.matmul(out=ps[:],
                                         lhsT=w1t[k][:, m * 128:(m + 1) * 128],
                                         rhs=h_t[:, k, :],
                                         start=(k == 0), stop=(k == KT1 - 1))
                    nc.vector.bn_stats(out=st1[m][:, t, :], in_=ps[:, :wvalid])
                    nc.scalar.activation(out=y1[m][t][:], in_=ps[:], func=ACopy)

            # ---------------- stats1 all-reduce + BN1 params ----------------
            sums1 = smallp.tile([128, MT, 2], F32, tag="sums1")
            mv_t = smallp.tile([128, 2], F32, tag="mv")
            tmp1 = smallp.tile([128, 1], F32, tag="tmp1")
            for m in range(MT):
                nc.vector.bn_aggr(out=mv_t[:], in_=st1[m][:])
                # sum = n*mean ; sumsq = n*(var + mean^2)
                nc.vector.tensor_mul(out=tmp1[:], in0=mv_t[:, 0:1], in1=mv_t[:, 0:1])
                nc.vector.tensor_add(out=tmp1[:], in0=tmp1[:], in1=mv_t[:, 1:2])
                nc.vector.tensor_scalar_mul(sums1[:, m, 0:1], mv_t[:, 0:1],
                                            float(NSH))
                nc.vector.tensor_scalar_mul(sums1[:, m, 1:2], tmp1[:], float(NSH))

            cc1_in = dramp.tile([128, MT * 2], F32, tag="cc1i")
            cc1_out = dramp.tile([128, MT * 2], F32, tag="cc1o")
            nc.sync.dma_start(out=cc1_in[:], in_=sums1[:].rearrange("p a b -> p (a b)"))
            nc.gpsimd.collective_compute(
                "AllReduce", AADD, replica_groups=[list(range(NCORES))],
                ins=[cc1_in[:].opt()], outs=[cc1_out[:].opt()])
            gst1 = smallp.tile([128, MT, 2], F32, tag="gst1")
            nc.sync.dma_start(out=gst1[:].rearrange("p a b -> p (a b)"), in_=cc1_out[:])

            sc1 = wp.tile([128, MT], F32, tag="sc1")
            sh1 = wp.tile([128, MT], F32, tag="sh1")
            mean_t = smallp.tile([128, 1], F32, tag="meant")
            var_t = smallp.tile([128, 1], F32, tag="vart")
            for m in range(MT):
                nc.vector.tensor_scalar_mul(mean_t[:], gst1[:, m, 0:1], 1.0 / N)
                nc.vector.tensor_scalar_mul(var_t[:], gst1[:, m, 1:2], 1.0 / N)
                nc.vector.tensor_mul(out=tmp1[:], in0=mean_t[:], in1=mean_t[:])
                nc.vector.tensor_tensor(out=var_t[:], in0=var_t[:], in1=tmp1[:], op=mybir.AluOpType.subtract)
                nc.vector.tensor_scalar_add(var_t[:], var_t[:], EPS)
                nc.scalar.activation(out=var_t[:], in_=var_t[:], func=ASqrt)
                nc.vector.reciprocal(out=var_t[:], in_=var_t[:])
                nc.vector.tensor_mul(out=sc1[:, m:m + 1], in0=g1_sb[:, m:m + 1],
                                     in1=var_t[:])
                nc.vector.tensor_mul(out=tmp1[:], in0=mean_t[:], in1=sc1[:, m:m + 1])
                nc.vector.tensor_tensor(out=sh1[:, m:m + 1], in0=be1_sb[:, m:m + 1],
                                        in1=tmp1[:], op=mybir.AluOpType.subtract)

            # ---------------- normalize y1 (in place) + GEMM2 + stats2 ----------
            y2d = dramp.tile([HS, NCOL], BF16, tag="y2d")
            for t in range(NT):
                wvalid = TW if t < NT - 1 else LASTW
                for m in range(MT):
                    nc.scalar.activation(out=y1[m][t][:], in_=y1[m][t][:],
                                         func=ARelu, bias=sh1[:, m:m + 1],
                                         scale=sc1[:, m:m + 1])
                for m in range(MT):
                    ps = psg.tile([128, TW], F32, space="PSUM", tag="psg")
                    for k in range(KT2):
                        nc.tensor.matmul(out=ps[:],
                                         lhsT=w2t[k][:, m * 128:(m + 1) * 128],
                                         rhs=y1[k][t][:],
                                         start=(k == 0), stop=(k == KT2 - 1))
                    nc.vector.bn_stats(out=st2[m][:, t, :], in_=ps[:, :wvalid])
                    ev = evp.tile([128, TW], BF16, tag="y2ev")
                    nc.scalar.activation(out=ev[:], in_=ps[:], func=ACopy)
                    nc.sync.dma_start(
                        out=y2d[m * 128:(m + 1) * 128, t * TW:(t + 1) * TW],
                        in_=ev[:])

            # ---------------- stats2 all-reduce + BN2 params ----------------
            sums2 = smallp.tile([128, MT, 2], F32, tag="sums2")
            for m in range(MT):
                nc.vector.bn_aggr(out=mv_t[:], in_=st2[m][:])
                nc.vector.tensor_mul(out=tmp1[:], in0=mv_t[:, 0:1], in1=mv_t[:, 0:1])
                nc.vector.tensor_add(out=tmp1[:], in0=tmp1[:], in1=mv_t[:, 1:2])
                nc.vector.tensor_scalar_mul(sums2[:, m, 0:1], mv_t[:, 0:1],
                                            float(NSH))
                nc.vector.tensor_scalar_mul(sums2[:, m, 1:2], tmp1[:], float(NSH))

            cc2_in = dramp.tile([128, MT * 2], F32, tag="cc2i")
            cc2_out = dramp.tile([128, MT * 2], F32, tag="cc2o")
            nc.sync.dma_start(out=cc2_in[:], in_=sums2[:].rearrange("p a b -> p (a b)"))
            nc.gpsimd.collective_compute(
                "AllReduce", AADD, replica_groups=[list(range(NCORES))],
                ins=[cc2_in[:].opt()], outs=[cc2_out[:].opt()])
            gst2 = smallp.tile([128, MT, 2], F32, tag="gst2")
            nc.sync.dma_start(out=gst2[:].rearrange("p a b -> p (a b)"), in_=cc2_out[:])

            sc2 = wp.tile([128, MT], F32, tag="sc2")
            sh2 = wp.tile([128, MT], F32, tag="sh2")
            for m in range(MT):
                nc.vector.tensor_scalar_mul(mean_t[:], gst2[:, m, 0:1], 1.0 / N)
                nc.vector.tensor_scalar_mul(var_t[:], gst2[:, m, 1:2], 1.0 / N)
                nc.vector.tensor_mul(out=tmp1[:], in0=mean_t[:], in1=mean_t[:])
                nc.vector.tensor_tensor(out=var_t[:], in0=var_t[:], in1=tmp1[:], op=mybir.AluOpType.subtract)
                nc.vector.tensor_scalar_add(var_t[:], var_t[:], EPS)
                nc.scalar.activation(out=var_t[:], in_=var_t[:], func=ASqrt)
                nc.vector.reciprocal(out=var_t[:], in_=var_t[:])
                nc.vector.tensor_mul(out=sc2[:, m:m + 1], in0=g2_sb[:, m:m + 1],
                                     in1=var_t[:])
                nc.vector.tensor_mul(out=tmp1[:], in0=mean_t[:], in1=sc2[:, m:m + 1])
                nc.vector.tensor_tensor(out=sh2[:, m:m + 1], in0=be2_sb[:, m:m + 1],
                                        in1=tmp1[:], op=mybir.AluOpType.subtract)

            # ---------------- final normalize -> f32 output ----------------
            for t in range(NT):
                for m in range(MT):
                    y2t = evp.tile([128, TW], BF16, tag="y2in")
                    nc.sync.dma_start(
                        out=y2t[:],
                        in_=y2d[m * 128:(m + 1) * 128, t * TW:(t + 1) * TW])
                    o32 = evp.tile([128, TW], F32, tag="o32")
                    nc.scalar.activation(out=o32[:], in_=y2t[:], func=AIdent,
                                         bias=sh2[:, m:m + 1],
                                         scale=sc2[:, m:m + 1])
                    nc.sync.dma_start(
                        out=t_out[m * 128:(m + 1) * 128, t * TW:(t + 1) * TW],
                        in_=o32[:])

    nc.compile()
    return nc


_CACHE = {}


def kernel(**inputs) -> np.ndarray:
    per_core, shared, perms, D, S = _host_prep(
        inputs["x"], inputs["edge_attr"], inputs["u"],
        inputs["w1"], inputs["w2"],
        inputs["g1"], inputs["be1"], inputs["g2"], inputs["be2"],
        inputs["edge_index"], inputs["batch"])

    key = (S, tuple(D))
    if key not in _CACHE:
        _CACHE[key] = _build(D, S)
    nc = _CACHE[key]

    in_maps = [{**per_core[c], **shared} for c in range(NCORES)]
    import os
    trace = bool(int(os.environ.get("KERNEL_TRACE", "0")))
    res = run_bass_kernel_spmd(nc, in_maps, core_ids=list(range(NCORES)),
                               trace=trace)
    if trace and res.exec_time_ns is not None:
        print(f"HW exec time: {res.exec_time_ns} ns")
        kernel.last_exec_time_ns = res.exec_time_ns

    out = np.empty((N, HS), np.float32)
    for c in range(NCORES):
        oT = res.results[c]["outT"]  # [HS, NCOL]
        blk = out[c * NSH:(c + 1) * NSH]
        blk[perms[c]] = oT[:, :NSH].T
    return out


# revision 7
# speedup vs baseline: 1.0043x; 1.0043x over previous
"""GNN NodeModel kernel for 8 Trainium2 NeuronCores (Bass/Tile).

Pipeline (per the reference nn.Module):
  scatter_max / scatter_mean / scatter_add of edge_attr by edge dest ->
  h = [x, u[batch], smax, smean, ssum]  (N x 832) ->
  Linear(832->1024) -> BatchNorm(train stats) -> ReLU ->
  Linear(1024->1024) -> BatchNorm(train stats)  => [N, 1024]

Sharding: nodes split into 8 contiguous shards of 6250; each core gets its
shard's incoming edges (bucketed by col on host).  Within a shard nodes are
degree-sorted and packed into 13 tiles of 512 (last 106 valid + padding).
Edges are laid out host-side in a padded ELL format so the device scatter is
a dense max/add accumulation.  All GEMMs run transposed (channels on
partitions, nodes on the free dim) in bf16 with fp32 PSUM accumulate; BN
statistics are computed per-channel with bn_stats and all-reduced across the
8 cores on-device.  BN biases b1/b2 cancel inside train-mode BatchNorm and
are not used.
"""

import numpy as np
import ml_dtypes

import concourse.bass as bass
import concourse.bacc as bacc
import concourse.tile as tile
from concourse import mybir
from concourse.bass_utils import run_bass_kernel_spmd

BF16 = mybir.dt.bfloat16
F32 = mybir.dt.float32

NCORES = 8
N = 50000
E = 800000
XI = 512
EI = 64
UI = 128
HS = 1024
G = 8
EPS = 1e-5
CIN = XI + 3 * EI + UI  # 832

NSH = N // NCORES          # 6250 nodes per core
TW = 512                   # node-tile width (free dim)
NT = 13                    # tiles per core (12*512 + 106)
NCOL = NT * TW             # 6656 padded columns
LASTW = NSH - (NT - 1) * TW  # 106
KT1 = 7                    # GEMM1 k-tiles (896 = 832 + 64 pad)
KT2 = 8                    # GEMM2 k-tiles
MT = HS // 128             # 8 channel tiles
NEG = -1000.0              # ELL pad for the max reduction


# ----------------------------------------------------------------------------
# Host-side sharding / layout prep
# ----------------------------------------------------------------------------

def _host_prep(x, edge_attr, u, w1, w2, g1, be1, g2, be2, edge_index, batch):
    bf = ml_dtypes.bfloat16
    col = np.asarray(edge_index[1])
    deg_all = np.bincount(col, minlength=N).astype(np.int64)

    shard_of_edge = col // NSH

    # per-core degree-sorted node order and per-tile slot counts
    perms = []
    degs_sorted = []
    for c in range(NCORES):
        dc = deg_all[c * NSH:(c + 1) * NSH]
        perm = np.argsort(-dc, kind="stable")
        perms.append(perm)
        degs_sorted.append(dc[perm])

    # global per-tile slot counts (same on every core so one NEFF fits all),
    # padded to a multiple of 4 for the pair-tree reduction
    D = []
    for t in range(NT):
        m = 0
        for c in range(NCORES):
            seg = degs_sorted[c][t * TW:(t + 1) * TW]
            if seg.size:
                m = max(m, int(seg.max()))
        D.append(-(-m // 4) * 4)
    offs = np.concatenate([[0], np.cumsum(D)]).astype(np.int64)
    S = int(offs[-1])

    per_core = []
    ea_bf = np.asarray(edge_attr, np.float32).astype(bf)
    x_f = np.asarray(x, np.float32)
    batch_np = np.asarray(batch)

    for c in range(NCORES):
        perm = perms[c]
        inv = np.empty(NSH, np.int64)
        inv[perm] = np.arange(NSH)

        emask = shard_of_edge == c
        l_orig = col[emask] - c * NSH          # local node id
        l = inv[l_orig]                         # degree-sorted local id
        vals = ea_bf[emask]                     # [Ec, 64] bf16

        order = np.argsort(l, kind="stable")
        l_s = l[order]
        vals_s = vals[order]
        first = np.searchsorted(l_s, l_s, side="left")
        slot = np.arange(l_s.size) - first      # rank within node

        t_arr = l_s // TW
        rem = l_s % TW
        g_arr = rem // 256
        j_arr = rem % 256
        s_glob = offs[t_arr] + slot

        ell_max = np.full((2, 64, S, 256), NEG, dtype=bf)
        ell_sum = np.zeros((2, 64, S, 256), dtype=bf)
        ell_max[g_arr, :, s_glob, j_arr] = vals_s
        ell_sum[g_arr, :, s_glob, j_arr] = vals_s

        # x^T [512, NCOL], permuted + zero-padded
        xT = np.zeros((XI, NCOL), dtype=bf)
        xT[:, :NSH] = x_f[c * NSH:(c + 1) * NSH][perm].T.astype(bf)

        # u one-hot [8, NCOL]
        onehot = np.zeros((G, NCOL), dtype=bf)
        bvals = batch_np[c * NSH:(c + 1) * NSH][perm]
        onehot[bvals, np.arange(NSH)] = bf(1.0)

        # per-node 1/max(deg,1) and (deg>0) mask, stacked-group layout:
        # per tile t: cols [t*512, t*512+256) = inv, [+256, +512) = mask
        dsort = degs_sorted[c].astype(np.float32)
        dpad = np.zeros(NCOL, np.float32)
        dpad[:NSH] = dsort
        dv = dpad.reshape(NT, 2, 256)
        ivmk = np.zeros((2, NT, 2, 256), np.float32)
        ivmk[:, :, 0, :] = (1.0 / np.maximum(dv, 1.0)).transpose(1, 0, 2)
        ivmk[:, :, 1, :] = (dv > 0).astype(np.float32).transpose(1, 0, 2)
        ivmk = ivmk.reshape(2, NT * 512).astype(bf)

        per_core.append(dict(
            xT=np.ascontiguousarray(xT),
            ellmax=np.ascontiguousarray(ell_max.reshape(128, S * 256)),
            ellsum=np.ascontiguousarray(ell_sum.reshape(128, S * 256)),
            onehot=np.ascontiguousarray(onehot),
            ivmk=np.ascontiguousarray(ivmk),
        ))

    # replicated weights
    w1 = np.asarray(w1, np.float32)
    w2 = np.asarray(w2, np.float32)
    w1T = np.zeros((KT1 * 128, HS), dtype=bf)
    w1T[0:512] = w1[:, 0:512].T.astype(bf)        # x block
    w1T[512:640] = w1[:, 512:640].T.astype(bf)    # u block
    w1T[640:704] = w1[:, 640:704].T.astype(bf)    # smax  (k5 top)
    w1T[704:768] = w1[:, 768:832].T.astype(bf)    # ssum  (k5 bottom)
    w1T[768:832] = w1[:, 704:768].T.astype(bf)    # smean (k6 top)
    w2T = np.ascontiguousarray(w2.T.astype(bf))

    def cvec(v):
        return np.ascontiguousarray(
            np.asarray(v, np.float32).reshape(MT, 128).T)

    shared = dict(
        w1T=np.ascontiguousarray(w1T),
        w2T=w2T,
        u8=np.asarray(u, np.float32).astype(bf),
        g1t=cvec(g1), be1t=cvec(be1), g2t=cvec(g2), be2t=cvec(be2),
    )
    return per_core, shared, perms, D, S


# ----------------------------------------------------------------------------
# Device kernel
# ----------------------------------------------------------------------------

def _build(D, S):
    nc = bacc.Bacc("TRN2", target_bir_lowering=False, debug=False,
                   num_devices=NCORES)

    t_xT = nc.dram_tensor("xT", [XI, NCOL], BF16, kind="ExternalInput")
    t_emax = nc.dram_tensor("ellmax", [128, S * 256], BF16, kind="ExternalInput")
    t_esum = nc.dram_tensor("ellsum", [128, S * 256], BF16, kind="ExternalInput")
    t_oneh = nc.dram_tensor("onehot", [G, NCOL], BF16, kind="ExternalInput")
    t_ivmk = nc.dram_tensor("ivmk", [2, NT * 512], BF16, kind="ExternalInput")
    t_u8 = nc.dram_tensor("u8", [G, UI], BF16, kind="ExternalInput")
    t_w1T = nc.dram_tensor("w1T", [KT1 * 128, HS], BF16, kind="ExternalInput")
    t_w2T = nc.dram_tensor("w2T", [HS, HS], BF16, kind="ExternalInput")
    t_g1 = nc.dram_tensor("g1t", [128, MT], F32, kind="ExternalInput")
    t_be1 = nc.dram_tensor("be1t", [128, MT], F32, kind="ExternalInput")
    t_g2 = nc.dram_tensor("g2t", [128, MT], F32, kind="ExternalInput")
    t_be2 = nc.dram_tensor("be2t", [128, MT], F32, kind="ExternalInput")
    t_out = nc.dram_tensor("outT", [HS, NCOL], F32, kind="ExternalOutput")

    offs = np.concatenate([[0], np.cumsum(D)]).astype(np.int64)
    AMAX = mybir.AluOpType.max
    AADD = mybir.AluOpType.add
    ACopy = mybir.ActivationFunctionType.Copy
    AIdent = mybir.ActivationFunctionType.Identity
    ARelu = mybir.ActivationFunctionType.Relu
    ASqrt = mybir.ActivationFunctionType.Sqrt

    with tile.TileContext(nc) as tc:
        with (
            tc.tile_pool(name="wp", bufs=1) as wp,
            tc.tile_pool(name="y1p", bufs=1) as y1p,
            tc.tile_pool(name="hp", bufs=3) as hp,
            tc.tile_pool(name="ellp", bufs=2) as ellp,
            tc.tile_pool(name="accp", bufs=2) as accp,
            tc.tile_pool(name="smallp", bufs=2) as smallp,
            tc.tile_pool(name="evp", bufs=2) as evp,
            tc.tile_pool(name="statp", bufs=1) as statp,
            tc.tile_pool(name="psg", bufs=5, space="PSUM") as psg,
            tc.tile_pool(name="psu", bufs=1, space="PSUM") as psu,
            tc.tile_pool(name="dramp", bufs=1, space="DRAM") as dramp,
        ):
            # ---- resident constants ----
            w1t = []
            for k in range(KT1):
                wt_ = wp.tile([128, HS], BF16, tag=f"w1_{k}")
                nc.sync.dma_start(out=wt_[:], in_=t_w1T[k * 128:(k + 1) * 128, :])
                w1t.append(wt_)
            w2t = []
            for k in range(KT2):
                wt_ = wp.tile([128, HS], BF16, tag=f"w2_{k}")
                nc.sync.dma_start(out=wt_[:], in_=t_w2T[k * 128:(k + 1) * 128, :])
                w2t.append(wt_)
            u_sb = wp.tile([G, UI], BF16, tag="u8")
            nc.sync.dma_start(out=u_sb[:], in_=t_u8[:])
            g1_sb = wp.tile([128, MT], F32, tag="g1")
            be1_sb = wp.tile([128, MT], F32, tag="be1")
            g2_sb = wp.tile([128, MT], F32, tag="g2")
            be2_sb = wp.tile([128, MT], F32, tag="be2")
            for tt, sb in ((t_g1, g1_sb), (t_be1, be1_sb),
                           (t_g2, g2_sb), (t_be2, be2_sb)):
                nc.sync.dma_start(out=sb[:], in_=tt[:])
            # selection matrix for broadcasting [2, x] rows to partition halves
            sel2_np = np.zeros((2, 128), dtype=ml_dtypes.bfloat16)
            sel2_np[0, 0:64] = 1.0
            sel2_np[1, 64:128] = 1.0
            sel2_dram = nc.inline_tensor(sel2_np, name="sel2c")
            sel2 = wp.tile([2, 128], BF16, tag="sel2")
            nc.sync.dma_start(out=sel2[:], in_=sel2_dram[:])

            y1 = [[y1p.tile([128, TW], BF16, tag=f"y1_{m}_{t}",
                            name=f"y1_{m}_{t}")
                   for t in range(NT)] for m in range(MT)]
            sY1 = [statp.tile([128, NT], F32, tag=f"sY1_{m}", name=f"sY1_{m}")
                   for m in range(MT)]
            sQ1 = [statp.tile([128, NT], F32, tag=f"sQ1_{m}", name=f"sQ1_{m}")
                   for m in range(MT)]
            sY2 = [statp.tile([128, NT], F32, tag=f"sY2_{m}", name=f"sY2_{m}")
                   for m in range(MT)]
            sQ2 = [statp.tile([128, NT], F32, tag=f"sQ2_{m}", name=f"sQ2_{m}")
                   for m in range(MT)]
            dump = statp.tile([128, TW], BF16, tag="dump")

            # ---------------- phase 1: scatter + GEMM1 + stats1 ----------------
            for t in range(NT):
                wvalid = TW if t < NT - 1 else LASTW
                h_t = hp.tile([128, KT1, TW], BF16, tag="h")
                for k in range(4):
                    nc.sync.dma_start(
                        out=h_t[:, k, :],
                        in_=t_xT[k * 128:(k + 1) * 128, t * TW:(t + 1) * TW])

                # u[batch] via one-hot matmul
                oh_t = smallp.tile([G, TW], BF16, tag="oh")
                nc.sync.dma_start(out=oh_t[:], in_=t_oneh[:, t * TW:(t + 1) * TW])
                ps_u = psu.tile([128, TW], F32, space="PSUM", tag="psu")
                nc.tensor.matmul(out=ps_u[:], lhsT=u_sb[:], rhs=oh_t[:],
                                 start=True, stop=True)
                nc.scalar.activation(out=h_t[:, 4, :], in_=ps_u[:], func=ACopy)

                # broadcast per-node inv/mask rows to both partition halves
                iv_t = smallp.tile([2, TW], BF16, tag="iv")
                nc.sync.dma_start(out=iv_t[:], in_=t_ivmk[:, t * TW:(t + 1) * TW])
                ps_b = psu.tile([128, TW], F32, space="PSUM", tag="psb")
                nc.tensor.matmul(out=ps_b[:], lhsT=sel2[:], rhs=iv_t[:],
                                 start=True, stop=True)

                # ELL scatter: pair-tree max / sum over D[t] slots
                ngr = D[t] // 4
                a2m = accp.tile([128, 2, 256], BF16, tag="a2m")
                a2s = accp.tile([128, 2, 256], BF16, tag="a2s")
                for gi in range(ngr):
                    base = (offs[t] + 4 * gi) * 256
                    cm = ellp.tile([128, 4, 256], BF16, tag="cm")
                    cs = ellp.tile([128, 4, 256], BF16, tag="cs")
                    nc.sync.dma_start(out=cm[:], in_=t_emax[:, base:base + 1024])
                    nc.sync.dma_start(out=cs[:], in_=t_esum[:, base:base + 1024])
                    if gi == 0:
                        nc.vector.tensor_tensor(out=a2m[:], in0=cm[:, 0:2, :],
                                                in1=cm[:, 2:4, :], op=AMAX)
                        nc.vector.tensor_tensor(out=a2s[:], in0=cs[:, 0:2, :],
                                                in1=cs[:, 2:4, :], op=AADD)
                    else:
                        nc.vector.tensor_tensor(out=a2m[:], in0=a2m[:],
                                                in1=cm[:, 0:2, :], op=AMAX)
                        nc.vector.tensor_tensor(out=a2m[:], in0=a2m[:],
                                                in1=cm[:, 2:4, :], op=AMAX)
                        nc.vector.tensor_tensor(out=a2s[:], in0=a2s[:],
                                                in1=cs[:, 0:2, :], op=AADD)
                        nc.vector.tensor_tensor(out=a2s[:], in0=a2s[:],
                                                in1=cs[:, 2:4, :], op=AADD)

                accm = accp.tile([128, 256], BF16, tag="accm")
                accs = accp.tile([128, 256], BF16, tag="accs")
                if ngr > 0:
                    nc.vector.tensor_tensor(out=accm[:], in0=a2m[:, 0, :],
                                            in1=a2m[:, 1, :], op=AMAX)
                    nc.vector.tensor_tensor(out=accs[:], in0=a2s[:, 0, :],
                                            in1=a2s[:, 1, :], op=AADD)
                else:
                    nc.gpsimd.memset(accm[:], 0.0)
                    nc.gpsimd.memset(accs[:], 0.0)

                # mask empty nodes; smean = ssum * inv
                nc.vector.tensor_mul(out=accm[:], in0=accm[:],
                                     in1=ps_b[:, 256:512])
                smean = accp.tile([128, 256], BF16, tag="smean")
                nc.vector.tensor_mul(out=smean[:], in0=accs[:],
                                     in1=ps_b[:, 0:256])

                # restack [2x64-feat groups, 256] -> [64-feat, 512] rows of h
                # k5 = [smax ; ssum], k6 = [smean ; 0]
                nc.vector.tensor_copy(out=h_t[0:64, 5, 0:256], in_=accm[0:64, :])
                nc.sync.dma_start(out=h_t[0:64, 5, 256:512], in_=accm[64:128, :])
                nc.sync.dma_start(out=h_t[64:128, 5, 0:256], in_=accs[0:64, :])
                nc.vector.tensor_copy(out=h_t[64:128, 5, 256:512], in_=accs[64:128, :])
                nc.vector.tensor_copy(out=h_t[0:64, 6, 0:256], in_=smean[0:64, :])
                nc.sync.dma_start(out=h_t[0:64, 6, 256:512], in_=smean[64:128, :])
                nc.gpsimd.memset(h_t[64:128, 6, :], 0.0)

                # GEMM1 + evac(+sum) + square(+sumsq)
                for m in range(MT):
                    ps = psg.tile([128, TW], F32, space="PSUM", tag="psg")
                    for k in range(KT1):
                        nc.tensor.matmul(out=ps[:],
                                         lhsT=w1t[k][:, m * 128:(m + 1) * 128],
                                         rhs=h_t[:, k, :],
                                         start=(k == 0), stop=(k == KT1 - 1))
                    nc.scalar.activation(out=y1[m][t][:], in_=ps[:], func=ACopy,
                                         accum_out=sY1[m][:, t:t + 1])
                    nc.vector.scalar_tensor_tensor(
                        out=dump[:], in0=y1[m][t][:], scalar=1.0,
                        in1=y1[m][t][:], op0=mybir.AluOpType.mult,
                        op1=mybir.AluOpType.mult,
                        accum_out=sQ1[m][:, t:t + 1])

            # ---------------- stats1 all-reduce + BN1 params ----------------
            sums1 = smallp.tile([128, MT, 2], F32, tag="sums1")
            tmp1 = smallp.tile([128, 1], F32, tag="tmp1")
            for m in range(MT):
                nc.vector.reduce_sum(sums1[:, m, 0:1], sY1[m][:], axis=mybir.AxisListType.X)
                nc.vector.reduce_sum(sums1[:, m, 1:2], sQ1[m][:], axis=mybir.AxisListType.X)

            cc1_in = dramp.tile([128, MT * 2], F32, tag="cc1i")
            cc1_out = dramp.tile([128, MT * 2], F32, tag="cc1o")
            nc.sync.dma_start(out=cc1_in[:], in_=sums1[:].rearrange("p a b -> p (a b)"))
            nc.gpsimd.collective_compute(
                "AllReduce", AADD, replica_groups=[list(range(NCORES))],
                ins=[cc1_in[:].opt()], outs=[cc1_out[:].opt()])
            gst1 = smallp.tile([128, MT, 2], F32, tag="gst1")
            nc.sync.dma_start(out=gst1[:].rearrange("p a b -> p (a b)"), in_=cc1_out[:])

            sc1 = wp.tile([128, MT], F32, tag="sc1")
            sh1 = wp.tile([128, MT], F32, tag="sh1")
            mean_t = smallp.tile([128, 1], F32, tag="meant")
            var_t = smallp.tile([128, 1], F32, tag="vart")
            for m in range(MT):
                nc.vector.tensor_scalar_mul(mean_t[:], gst1[:, m, 0:1], 1.0 / N)
                nc.vector.tensor_scalar_mul(var_t[:], gst1[:, m, 1:2], 1.0 / N)
                nc.vector.tensor_mul(out=tmp1[:], in0=mean_t[:], in1=mean_t[:])
                nc.vector.tensor_tensor(out=var_t[:], in0=var_t[:], in1=tmp1[:], op=mybir.AluOpType.subtract)
                nc.vector.tensor_scalar_add(var_t[:], var_t[:], EPS)
                nc.scalar.activation(out=var_t[:], in_=var_t[:], func=ASqrt)
                nc.vector.reciprocal(out=var_t[:], in_=var_t[:])
                nc.vector.tensor_mul(out=sc1[:, m:m + 1], in0=g1_sb[:, m:m + 1],
                                     in1=var_t[:])
                nc.vector.tensor_mul(out=tmp1[:], in0=mean_t[:], in1=sc1[:, m:m + 1])
                nc.vector.tensor_tensor(out=sh1[:, m:m + 1], in0=be1_sb[:, m:m + 1],
                                        in1=tmp1[:], op=mybir.AluOpType.subtract)

            # ---------------- normalize y1 (in place) + GEMM2 + stats2 ----------
            y2d = dramp.tile([HS, NCOL], BF16, tag="y2d")
            for t in range(NT):
                wvalid = TW if t < NT - 1 else LASTW
                for m in range(MT):
                    nc.scalar.activation(out=y1[m][t][:], in_=y1[m][t][:],
                                         func=ARelu, bias=sh1[:, m:m + 1],
                                         scale=sc1[:, m:m + 1])
                    if t == NT - 1:
                        nc.gpsimd.memset(y1[m][t][:, LASTW:], 0.0)
                for m in range(MT):
                    ps = psg.tile([128, TW], F32, space="PSUM", tag="psg")
                    for k in range(KT2):
                        nc.tensor.matmul(out=ps[:],
                                         lhsT=w2t[k][:, m * 128:(m + 1) * 128],
                                         rhs=y1[k][t][:],
                                         start=(k == 0), stop=(k == KT2 - 1))
                    ev = evp.tile([128, TW], BF16, tag="y2ev")
                    nc.scalar.activation(out=ev[:], in_=ps[:], func=ACopy,
                                         accum_out=sY2[m][:, t:t + 1])
                    nc.vector.scalar_tensor_tensor(
                        out=dump[:], in0=ev[:], scalar=1.0,
                        in1=ev[:], op0=mybir.AluOpType.mult,
                        op1=mybir.AluOpType.mult,
                        accum_out=sQ2[m][:, t:t + 1])
                    nc.sync.dma_start(
                        out=y2d[m * 128:(m + 1) * 128, t * TW:(t + 1) * TW],
                        in_=ev[:])

            # ---------------- stats2 all-reduce + BN2 params ----------------
            sums2 = smallp.tile([128, MT, 2], F32, tag="sums2")
            for m in range(MT):
                nc.vector.reduce_sum(sums2[:, m, 0:1], sY2[m][:], axis=mybir.AxisListType.X)
                nc.vector.reduce_sum(sums2[:, m, 1:2], sQ2[m][:], axis=mybir.AxisListType.X)

            cc2_in = dramp.tile([128, MT * 2], F32, tag="cc2i")
            cc2_out = dramp.tile([128, MT * 2], F32, tag="cc2o")
            nc.sync.dma_start(out=cc2_in[:], in_=sums2[:].rearrange("p a b -> p (a b)"))
            nc.gpsimd.collective_compute(
                "AllReduce", AADD, replica_groups=[list(range(NCORES))],
                ins=[cc2_in[:].opt()], outs=[cc2_out[:].opt()])
            gst2 = smallp.tile([128, MT, 2], F32, tag="gst2")
            nc.sync.dma_start(out=gst2[:].rearrange("p a b -> p (a b)"), in_=cc2_out[:])

            sc2 = wp.tile([128, MT], F32, tag="sc2")
            sh2 = wp.tile([128, MT], F32, tag="sh2")
            for m in range(MT):
                nc.vector.tensor_scalar_mul(mean_t[:], gst2[:, m, 0:1], 1.0 / N)
                nc.vector.tensor_scalar_mul(var_t[:], gst2[:, m, 1:2], 1.0 / N)
                nc.vector.tensor_mul(out=tmp1[:], in0=mean_t[:], in1=mean_t[:])
                nc.vector.tensor_tensor(out=var_t[:], in0=var_t[:], in1=tmp1[:], op=mybir.AluOpType.subtract)
                nc.vector.tensor_scalar_add(var_t[:], var_t[:], EPS)
                nc.scalar.activation(out=var_t[:], in_=var_t[:], func=ASqrt)
                nc.vector.reciprocal(out=var_t[:], in_=var_t[:])
                nc.vector.tensor_mul(out=sc2[:, m:m + 1], in0=g2_sb[:, m:m + 1],
                                     in1=var_t[:])
                nc.vector.tensor_mul(out=tmp1[:], in0=mean_t[:], in1=sc2[:, m:m + 1])
                nc.vector.tensor_tensor(out=sh2[:, m:m + 1], in0=be2_sb[:, m:m + 1],
                                        in1=tmp1[:], op=mybir.AluOpType.subtract)

            # ---------------- final normalize -> f32 output ----------------
            for t in range(NT):
                for m in range(MT):
                    y2t = evp.tile([128, TW], BF16, tag="y2in")
                    nc.sync.dma_start(
                        out=y2t[:],
                        in_=y2d[m * 128:(m + 1) * 128, t * TW:(t + 1) * TW])
                    o32 = evp.tile([128, TW], F32, tag="o32")
                    nc.vector.tensor_scalar(o32[:], y2t[:], sc2[:, m:m + 1],
                                            sh2[:, m:m + 1],
                                            mybir.AluOpType.mult, AADD)
                    nc.sync.dma_start(
                        out=t_out[m * 128:(m + 1) * 128, t * TW:(t + 1) * TW],
                        in_=o32[:])

    nc.compile()
    return nc


_CACHE = {}


def kernel(**inputs) -> np.ndarray:
    per_core, shared, perms, D, S = _host_prep(
        inputs["x"], inputs["edge_attr"], inputs["u"],
        inputs["w1"], inputs["w2"],
        inputs["g1"], inputs["be1"], inputs["g2"], inputs["be2"],
        inputs["edge_index"], inputs["batch"])

    key = (S, tuple(D))
    if key not in _CACHE:
        _CACHE[key] = _build(D, S)
    nc = _CACHE[key]

    in_maps = [{**per_core[c], **shared} for c in range(NCORES)]
    import os
    trace = bool(int(os.environ.get("KERNEL_TRACE", "0")))
    res = run_bass_kernel_spmd(nc, in_maps, core_ids=list(range(NCORES)),
                               trace=trace)
    if trace and res.exec_time_ns is not None:
        print(f"HW exec time: {res.exec_time_ns} ns")
        kernel.last_exec_time_ns = res.exec_time_ns

    out = np.empty((N, HS), np.float32)
    for c in range(NCORES):
        oT = res.results[c]["outT"]  # [HS, NCOL]
        blk = out[c * NSH:(c + 1) * NSH]
        blk[perms[c]] = oT[:, :NSH].T
    return out


# revision 9
# speedup vs baseline: 1.1837x; 1.1786x over previous
"""GNN NodeModel kernel for 8 Trainium2 NeuronCores (Bass/Tile).

Pipeline (per the reference nn.Module):
  scatter_max / scatter_mean / scatter_add of edge_attr by edge dest ->
  h = [x, u[batch], smax, smean, ssum]  (N x 832) ->
  Linear(832->1024) -> BatchNorm(train stats) -> ReLU ->
  Linear(1024->1024) -> BatchNorm(train stats)  => [N, 1024]

Sharding: nodes split into 8 contiguous shards of 6250; each core gets its
shard's incoming edges (bucketed by col on host).  Within a shard nodes are
degree-sorted and packed into 13 tiles of 512 (last 106 valid + padding).
Edges are laid out host-side in a padded ELL format so the device scatter is
a dense max/add accumulation.  All GEMMs run transposed (channels on
partitions, nodes on the free dim) in bf16 with fp32 PSUM accumulate; BN
statistics are computed per-channel with bn_stats and all-reduced across the
8 cores on-device.  BN biases b1/b2 cancel inside train-mode BatchNorm and
are not used.
"""

import numpy as np
import ml_dtypes

import concourse.bass as bass
import concourse.bacc as bacc
import concourse.tile as tile
from concourse import mybir
from concourse.bass_utils import run_bass_kernel_spmd

BF16 = mybir.dt.bfloat16
F32 = mybir.dt.float32

NCORES = 8
N = 50000
E = 800000
XI = 512
EI = 64
UI = 128
HS = 1024
G = 8
EPS = 1e-5
CIN = XI + 3 * EI + UI  # 832

NSH = N // NCORES          # 6250 nodes per core
TW = 512                   # node-tile width (free dim)
NT = 13                    # tiles per core (12*512 + 106)
NCOL = NT * TW             # 6656 padded columns
LASTW = NSH - (NT - 1) * TW  # 106
KT1 = 7                    # GEMM1 k-tiles (896 = 832 + 64 pad)
KT2 = 8                    # GEMM2 k-tiles
MT = HS // 128             # 8 channel tiles
NEG = -1000.0              # ELL pad for the max reduction


# ----------------------------------------------------------------------------
# Host-side sharding / layout prep
# ----------------------------------------------------------------------------

def _host_prep(x, edge_attr, u, w1, w2, g1, be1, g2, be2, edge_index, batch):
    bf = ml_dtypes.bfloat16
    col = np.asarray(edge_index[1])
    deg_all = np.bincount(col, minlength=N).astype(np.int64)

    shard_of_edge = col // NSH

    # per-core degree-sorted node order and per-tile slot counts
    perms = []
    degs_sorted = []
    for c in range(NCORES):
        dc = deg_all[c * NSH:(c + 1) * NSH]
        perm = np.argsort(-dc, kind="stable")
        perms.append(perm)
        degs_sorted.append(dc[perm])

    # global per-tile slot counts (same on every core so one NEFF fits all),
    # padded to a multiple of 4 for the pair-tree reduction
    D = []
    for t in range(NT):
        m = 0
        for c in range(NCORES):
            seg = degs_sorted[c][t * TW:(t + 1) * TW]
            if seg.size:
                m = max(m, int(seg.max()))
        D.append(-(-m // 4) * 4)
    offs = np.concatenate([[0], np.cumsum(D)]).astype(np.int64)
    S = int(offs[-1])

    per_core = []
    ea_bf = np.asarray(edge_attr, np.float32).astype(bf)
    x_f = np.asarray(x, np.float32)
    batch_np = np.asarray(batch)

    for c in range(NCORES):
        perm = perms[c]
        inv = np.empty(NSH, np.int64)
        inv[perm] = np.arange(NSH)

        emask = shard_of_edge == c
        l_orig = col[emask] - c * NSH          # local node id
        l = inv[l_orig]                         # degree-sorted local id
        vals = ea_bf[emask]                     # [Ec, 64] bf16

        order = np.argsort(l, kind="stable")
        l_s = l[order]
        vals_s = vals[order]
        first = np.searchsorted(l_s, l_s, side="left")
        slot = np.arange(l_s.size) - first      # rank within node

        t_arr = l_s // TW
        rem = l_s % TW
        g_arr = rem // 256
        j_arr = rem % 256
        s_glob = offs[t_arr] + slot

        ell_max = np.full((2, 64, S, 256), NEG, dtype=bf)
        ell_sum = np.zeros((2, 64, S, 256), dtype=bf)
        ell_max[g_arr, :, s_glob, j_arr] = vals_s
        ell_sum[g_arr, :, s_glob, j_arr] = vals_s

        # x^T [512, NCOL], permuted + zero-padded
        xT = np.zeros((XI, NCOL), dtype=bf)
        xT[:, :NSH] = x_f[c * NSH:(c + 1) * NSH][perm].T.astype(bf)

        # u one-hot [8, NCOL]
        onehot = np.zeros((G, NCOL), dtype=bf)
        bvals = batch_np[c * NSH:(c + 1) * NSH][perm]
        onehot[bvals, np.arange(NSH)] = bf(1.0)

        # per-node 1/max(deg,1) and (deg>0) mask, stacked-group layout:
        # per tile t: cols [t*512, t*512+256) = inv, [+256, +512) = mask
        dsort = degs_sorted[c].astype(np.float32)
        dpad = np.zeros(NCOL, np.float32)
        dpad[:NSH] = dsort
        dv = dpad.reshape(NT, 2, 256)
        ivmk = np.zeros((2, NT, 2, 256), np.float32)
        ivmk[:, :, 0, :] = (1.0 / np.maximum(dv, 1.0)).transpose(1, 0, 2)
        ivmk[:, :, 1, :] = (dv > 0).astype(np.float32).transpose(1, 0, 2)
        ivmk = ivmk.reshape(2, NT * 512).astype(bf)

        per_core.append(dict(
            xT=np.ascontiguousarray(xT),
            ellmax=np.ascontiguousarray(ell_max.reshape(128, S * 256)),
            ellsum=np.ascontiguousarray(ell_sum.reshape(128, S * 256)),
            onehot=np.ascontiguousarray(onehot),
            ivmk=np.ascontiguousarray(ivmk),
        ))

    # replicated weights
    w1 = np.asarray(w1, np.float32)
    w2 = np.asarray(w2, np.float32)
    w1T = np.zeros((KT1 * 128, HS), dtype=bf)
    w1T[0:512] = w1[:, 0:512].T.astype(bf)        # x block
    w1T[512:640] = w1[:, 512:640].T.astype(bf)    # u block
    w1T[640:704] = w1[:, 640:704].T.astype(bf)    # smax  (k5 top)
    w1T[704:768] = w1[:, 768:832].T.astype(bf)    # ssum  (k5 bottom)
    w1T[768:832] = w1[:, 704:768].T.astype(bf)    # smean (k6 top)
    w2T = np.ascontiguousarray(w2.T.astype(bf))

    def cvec(v):
        return np.ascontiguousarray(
            np.asarray(v, np.float32).reshape(MT, 128).T)

    shared = dict(
        w1T=np.ascontiguousarray(w1T),
        w2T=w2T,
        u8=np.asarray(u, np.float32).astype(bf),
        g1t=cvec(g1), be1t=cvec(be1), g2t=cvec(g2), be2t=cvec(be2),
    )
    return per_core, shared, perms, D, S


# ----------------------------------------------------------------------------
# Device kernel
# ----------------------------------------------------------------------------

def _build(D, S):
    nc = bacc.Bacc("TRN2", target_bir_lowering=False, debug=False,
                   num_devices=NCORES)

    t_xT = nc.dram_tensor("xT", [XI, NCOL], BF16, kind="ExternalInput")
    t_emax = nc.dram_tensor("ellmax", [128, S * 256], BF16, kind="ExternalInput")
    t_esum = nc.dram_tensor("ellsum", [128, S * 256], BF16, kind="ExternalInput")
    t_oneh = nc.dram_tensor("onehot", [G, NCOL], BF16, kind="ExternalInput")
    t_ivmk = nc.dram_tensor("ivmk", [2, NT * 512], BF16, kind="ExternalInput")
    t_u8 = nc.dram_tensor("u8", [G, UI], BF16, kind="ExternalInput")
    t_w1T = nc.dram_tensor("w1T", [KT1 * 128, HS], BF16, kind="ExternalInput")
    t_w2T = nc.dram_tensor("w2T", [HS, HS], BF16, kind="ExternalInput")
    t_g1 = nc.dram_tensor("g1t", [128, MT], F32, kind="ExternalInput")
    t_be1 = nc.dram_tensor("be1t", [128, MT], F32, kind="ExternalInput")
    t_g2 = nc.dram_tensor("g2t", [128, MT], F32, kind="ExternalInput")
    t_be2 = nc.dram_tensor("be2t", [128, MT], F32, kind="ExternalInput")
    t_out = nc.dram_tensor("outT", [HS, NCOL], BF16, kind="ExternalOutput")

    offs = np.concatenate([[0], np.cumsum(D)]).astype(np.int64)
    AMAX = mybir.AluOpType.max
    AADD = mybir.AluOpType.add
    ACopy = mybir.ActivationFunctionType.Copy
    AIdent = mybir.ActivationFunctionType.Identity
    ARelu = mybir.ActivationFunctionType.Relu
    ASqrt = mybir.ActivationFunctionType.Sqrt

    with tile.TileContext(nc) as tc:
        with (
            tc.tile_pool(name="wp", bufs=1) as wp,
            tc.tile_pool(name="y1p", bufs=1) as y1p,
            tc.tile_pool(name="hp", bufs=3) as hp,
            tc.tile_pool(name="ellp", bufs=2) as ellp,
            tc.tile_pool(name="accp", bufs=2) as accp,
            tc.tile_pool(name="smallp", bufs=2) as smallp,
            tc.tile_pool(name="evp", bufs=2) as evp,
            tc.tile_pool(name="statp", bufs=1) as statp,
            tc.tile_pool(name="psg", bufs=1, space="PSUM") as psg,
            tc.tile_pool(name="psu", bufs=1, space="PSUM") as psu,
            tc.tile_pool(name="dramp", bufs=1, space="DRAM") as dramp,
        ):
            # ---- resident constants ----
            w1t = []
            for k in range(KT1):
                wt_ = wp.tile([128, HS], BF16, tag=f"w1_{k}")
                nc.sync.dma_start(out=wt_[:], in_=t_w1T[k * 128:(k + 1) * 128, :])
                w1t.append(wt_)
            w2t = []
            for k in range(KT2):
                wt_ = wp.tile([128, HS], BF16, tag=f"w2_{k}")
                nc.sync.dma_start(out=wt_[:], in_=t_w2T[k * 128:(k + 1) * 128, :])
                w2t.append(wt_)
            u_sb = wp.tile([G, UI], BF16, tag="u8")
            nc.sync.dma_start(out=u_sb[:], in_=t_u8[:])
            g1_sb = wp.tile([128, MT], F32, tag="g1")
            be1_sb = wp.tile([128, MT], F32, tag="be1")
            g2_sb = wp.tile([128, MT], F32, tag="g2")
            be2_sb = wp.tile([128, MT], F32, tag="be2")
            for tt, sb in ((t_g1, g1_sb), (t_be1, be1_sb),
                           (t_g2, g2_sb), (t_be2, be2_sb)):
                nc.sync.dma_start(out=sb[:], in_=tt[:])

            y1 = [[y1p.tile([128, TW], BF16, tag=f"y1_{m}_{t}",
                            name=f"y1_{m}_{t}")
                   for t in range(NT)] for m in range(MT)]
            sY1 = [statp.tile([128, NT], F32, tag=f"sY1_{m}", name=f"sY1_{m}")
                   for m in range(MT)]
            sQ1 = [statp.tile([128, NT], F32, tag=f"sQ1_{m}", name=f"sQ1_{m}")
                   for m in range(MT)]
            sY2 = [statp.tile([128, NT], F32, tag=f"sY2_{m}", name=f"sY2_{m}")
                   for m in range(MT)]
            sQ2 = [statp.tile([128, NT], F32, tag=f"sQ2_{m}", name=f"sQ2_{m}")
                   for m in range(MT)]
            dump = statp.tile([128, TW], BF16, tag="dump")

            # ---------------- phase 1: scatter + GEMM1 + stats1 ----------------
            for t in range(NT):
                wvalid = TW if t < NT - 1 else LASTW
                h_t = hp.tile([128, KT1, TW], BF16, tag="h")
                for k in range(4):
                    nc.sync.dma_start(
                        out=h_t[:, k, :],
                        in_=t_xT[k * 128:(k + 1) * 128, t * TW:(t + 1) * TW])

                # u[batch] via one-hot matmul
                oh_t = smallp.tile([G, TW], BF16, tag="oh")
                nc.sync.dma_start(out=oh_t[:], in_=t_oneh[:, t * TW:(t + 1) * TW])
                ps_u = psu.tile([128, TW], F32, space="PSUM", tag="psu")
                nc.tensor.matmul(out=ps_u[:], lhsT=u_sb[:], rhs=oh_t[:],
                                 start=True, stop=True)
                nc.scalar.activation(out=h_t[:, 4, :], in_=ps_u[:], func=ACopy)

                # broadcast per-node inv/mask rows to both partition halves
                a0 = t * TW
                ivb = evp.tile([128, TW], BF16, tag="ivb")
                nc.sync.dma_start(out=ivb[0:64, 0:256],
                                  in_=t_ivmk[0:1, a0:a0 + 256].to_broadcast([64, 256]))
                nc.sync.dma_start(out=ivb[64:128, 0:256],
                                  in_=t_ivmk[1:2, a0:a0 + 256].to_broadcast([64, 256]))
                nc.sync.dma_start(out=ivb[0:64, 256:512],
                                  in_=t_ivmk[0:1, a0 + 256:a0 + 512].to_broadcast([64, 256]))
                nc.sync.dma_start(out=ivb[64:128, 256:512],
                                  in_=t_ivmk[1:2, a0 + 256:a0 + 512].to_broadcast([64, 256]))

                # ELL scatter: pair-tree max / sum over D[t] slots
                ngr = D[t] // 4
                a2m = accp.tile([128, 2, 256], BF16, tag="a2m")
                a2s = accp.tile([128, 2, 256], BF16, tag="a2s")
                for gi in range(ngr):
                    base = (offs[t] + 4 * gi) * 256
                    cm = ellp.tile([128, 4, 256], BF16, tag="cm")
                    cs = ellp.tile([128, 4, 256], BF16, tag="cs")
                    nc.sync.dma_start(out=cm[:], in_=t_emax[:, base:base + 1024])
                    nc.sync.dma_start(out=cs[:], in_=t_esum[:, base:base + 1024])
                    if gi == 0:
                        nc.vector.tensor_tensor(out=a2m[:], in0=cm[:, 0:2, :],
                                                in1=cm[:, 2:4, :], op=AMAX)
                        nc.vector.tensor_tensor(out=a2s[:], in0=cs[:, 0:2, :],
                                                in1=cs[:, 2:4, :], op=AADD)
                    else:
                        nc.vector.tensor_tensor(out=a2m[:], in0=a2m[:],
                                                in1=cm[:, 0:2, :], op=AMAX)
                        nc.vector.tensor_tensor(out=a2m[:], in0=a2m[:],
                                                in1=cm[:, 2:4, :], op=AMAX)
                        nc.vector.tensor_tensor(out=a2s[:], in0=a2s[:],
                                                in1=cs[:, 0:2, :], op=AADD)
                        nc.vector.tensor_tensor(out=a2s[:], in0=a2s[:],
                                                in1=cs[:, 2:4, :], op=AADD)

                accm = accp.tile([128, 256], BF16, tag="accm")
                accs = accp.tile([128, 256], BF16, tag="accs")
                if ngr > 0:
                    nc.vector.tensor_tensor(out=accm[:], in0=a2m[:, 0, :],
                                            in1=a2m[:, 1, :], op=AMAX)
                    nc.vector.tensor_tensor(out=accs[:], in0=a2s[:, 0, :],
                                            in1=a2s[:, 1, :], op=AADD)
                else:
                    nc.gpsimd.memset(accm[:], 0.0)
                    nc.gpsimd.memset(accs[:], 0.0)

                # mask empty nodes; smean = ssum * inv
                nc.vector.tensor_mul(out=accm[:], in0=accm[:],
                                     in1=ivb[:, 256:512])
                smean = accp.tile([128, 256], BF16, tag="smean")
                nc.vector.tensor_mul(out=smean[:], in0=accs[:],
                                     in1=ivb[:, 0:256])

                # restack [2x64-feat groups, 256] -> [64-feat, 512] rows of h
                # k5 = [smax ; ssum], k6 = [smean ; 0]
                nc.vector.tensor_copy(out=h_t[0:64, 5, 0:256], in_=accm[0:64, :])
                nc.sync.dma_start(out=h_t[0:64, 5, 256:512], in_=accm[64:128, :])
                nc.sync.dma_start(out=h_t[64:128, 5, 0:256], in_=accs[0:64, :])
                nc.vector.tensor_copy(out=h_t[64:128, 5, 256:512], in_=accs[64:128, :])
                nc.vector.tensor_copy(out=h_t[0:64, 6, 0:256], in_=smean[0:64, :])
                nc.sync.dma_start(out=h_t[0:64, 6, 256:512], in_=smean[64:128, :])
                nc.gpsimd.memset(h_t[64:128, 6, :], 0.0)

                # GEMM1 + evac(+sum) + square(+sumsq); k-major inside
                # m-blocks of 4 so the scatter/restack (k=5,6) gets slack
                for mb in range(0, MT, 4):
                    blk = list(range(mb, mb + 4))
                    pss = {}
                    for m in blk:
                        pss[m] = psg.tile([128, TW], F32, space="PSUM",
                                          tag=f"psg{m % 4}", name=f"ps{m % 4}",
                                          bufs=1)
                    for k in range(KT1):
                        for m in blk:
                            nc.tensor.matmul(out=pss[m][:],
                                             lhsT=w1t[k][:, m * 128:(m + 1) * 128],
                                             rhs=h_t[:, k, :],
                                             start=(k == 0), stop=(k == KT1 - 1))
                    for m in blk:
                        nc.scalar.activation(out=y1[m][t][:], in_=pss[m][:],
                                             func=ACopy,
                                             accum_out=sY1[m][:, t:t + 1])
                        nc.vector.scalar_tensor_tensor(
                            out=dump[:], in0=y1[m][t][:], scalar=1.0,
                            in1=y1[m][t][:], op0=mybir.AluOpType.mult,
                            op1=mybir.AluOpType.mult,
                            accum_out=sQ1[m][:, t:t + 1])

            # ---------------- stats1 all-reduce + BN1 params ----------------
            sums1 = smallp.tile([128, MT, 2], F32, tag="sums1")
            tmp1 = smallp.tile([128, 1], F32, tag="tmp1")
            for m in range(MT):
                nc.vector.reduce_sum(sums1[:, m, 0:1], sY1[m][:], axis=mybir.AxisListType.X)
                nc.vector.reduce_sum(sums1[:, m, 1:2], sQ1[m][:], axis=mybir.AxisListType.X)

            cc1_in = dramp.tile([128, MT * 2], F32, tag="cc1i")
            cc1_out = dramp.tile([128, MT * 2], F32, tag="cc1o")
            nc.sync.dma_start(out=cc1_in[:], in_=sums1[:].rearrange("p a b -> p (a b)"))
            nc.gpsimd.collective_compute(
                "AllReduce", AADD, replica_groups=[list(range(NCORES))],
                ins=[cc1_in[:].opt()], outs=[cc1_out[:].opt()])
            gst1 = smallp.tile([128, MT, 2], F32, tag="gst1")
            nc.sync.dma_start(out=gst1[:].rearrange("p a b -> p (a b)"), in_=cc1_out[:])

            sc1 = wp.tile([128, MT], F32, tag="sc1")
            sh1 = wp.tile([128, MT], F32, tag="sh1")
            mean_t = smallp.tile([128, MT], F32, tag="meant")
            var_t = smallp.tile([128, MT], F32, tag="vart")
            tmp8 = smallp.tile([128, MT], F32, tag="tmp8")
            nc.vector.tensor_scalar_mul(mean_t[:], gst1[:, :, 0], 1.0 / N)
            nc.vector.tensor_scalar_mul(var_t[:], gst1[:, :, 1], 1.0 / N)
            nc.vector.tensor_mul(out=tmp8[:], in0=mean_t[:], in1=mean_t[:])
            nc.vector.tensor_tensor(out=var_t[:], in0=var_t[:], in1=tmp8[:],
                                    op=mybir.AluOpType.subtract)
            nc.vector.tensor_scalar_add(var_t[:], var_t[:], EPS)
            nc.scalar.activation(out=var_t[:], in_=var_t[:], func=ASqrt)
            nc.vector.reciprocal(out=var_t[:], in_=var_t[:])
            nc.vector.tensor_mul(out=sc1[:], in0=g1_sb[:], in1=var_t[:])
            nc.vector.tensor_mul(out=tmp8[:], in0=mean_t[:], in1=sc1[:])
            nc.vector.tensor_tensor(out=sh1[:], in0=be1_sb[:], in1=tmp8[:],
                                    op=mybir.AluOpType.subtract)

            # ---------------- normalize y1 (in place) + GEMM2 + stats2 ----------
            y2d = dramp.tile([HS, NCOL], BF16, tag="y2d")
            for t in range(NT):
                wvalid = TW if t < NT - 1 else LASTW
                for m in range(MT):
                    nc.scalar.activation(out=y1[m][t][:], in_=y1[m][t][:],
                                         func=ARelu, bias=sh1[:, m:m + 1],
                                         scale=sc1[:, m:m + 1])
                    if t == NT - 1:
                        nc.gpsimd.memset(y1[m][t][:, LASTW:], 0.0)
                for m in range(MT):
                    ps = psg.tile([128, TW], F32, space="PSUM", tag="psg",
                                  bufs=3)
                    for k in range(KT2):
                        nc.tensor.matmul(out=ps[:],
                                         lhsT=w2t[k][:, m * 128:(m + 1) * 128],
                                         rhs=y1[k][t][:],
                                         start=(k == 0), stop=(k == KT2 - 1))
                    ev = evp.tile([128, TW], BF16, tag="y2ev")
                    nc.scalar.activation(out=ev[:], in_=ps[:], func=ACopy,
                                         accum_out=sY2[m][:, t:t + 1])
                    nc.vector.scalar_tensor_tensor(
                        out=dump[:], in0=ev[:], scalar=1.0,
                        in1=ev[:], op0=mybir.AluOpType.mult,
                        op1=mybir.AluOpType.mult,
                        accum_out=sQ2[m][:, t:t + 1])
                    nc.sync.dma_start(
                        out=y2d[m * 128:(m + 1) * 128, t * TW:(t + 1) * TW],
                        in_=ev[:])

            # ---------------- stats2 all-reduce + BN2 params ----------------
            sums2 = smallp.tile([128, MT, 2], F32, tag="sums2")
            for m in range(MT):
                nc.vector.reduce_sum(sums2[:, m, 0:1], sY2[m][:], axis=mybir.AxisListType.X)
                nc.vector.reduce_sum(sums2[:, m, 1:2], sQ2[m][:], axis=mybir.AxisListType.X)

            cc2_in = dramp.tile([128, MT * 2], F32, tag="cc2i")
            cc2_out = dramp.tile([128, MT * 2], F32, tag="cc2o")
            nc.sync.dma_start(out=cc2_in[:], in_=sums2[:].rearrange("p a b -> p (a b)"))
            nc.gpsimd.collective_compute(
                "AllReduce", AADD, replica_groups=[list(range(NCORES))],
                ins=[cc2_in[:].opt()], outs=[cc2_out[:].opt()])
            gst2 = smallp.tile([128, MT, 2], F32, tag="gst2")
            nc.sync.dma_start(out=gst2[:].rearrange("p a b -> p (a b)"), in_=cc2_out[:])

            sc2 = wp.tile([128, MT], F32, tag="sc2")
            sh2 = wp.tile([128, MT], F32, tag="sh2")
            nc.vector.tensor_scalar_mul(mean_t[:], gst2[:, :, 0], 1.0 / N)
            nc.vector.tensor_scalar_mul(var_t[:], gst2[:, :, 1], 1.0 / N)
            nc.vector.tensor_mul(out=tmp8[:], in0=mean_t[:], in1=mean_t[:])
            nc.vector.tensor_tensor(out=var_t[:], in0=var_t[:], in1=tmp8[:],
                                    op=mybir.AluOpType.subtract)
            nc.vector.tensor_scalar_add(var_t[:], var_t[:], EPS)
            nc.scalar.activation(out=var_t[:], in_=var_t[:], func=ASqrt)
            nc.vector.reciprocal(out=var_t[:], in_=var_t[:])
            nc.vector.tensor_mul(out=sc2[:], in0=g2_sb[:], in1=var_t[:])
            nc.vector.tensor_mul(out=tmp8[:], in0=mean_t[:], in1=sc2[:])
            nc.vector.tensor_tensor(out=sh2[:], in0=be2_sb[:], in1=tmp8[:],
                                    op=mybir.AluOpType.subtract)

            # ---------------- final normalize -> bf16 output ----------------
            CW = 832  # chunk width; NCOL = 8 * CW
            for m in range(MT):
                for ci in range(NCOL // CW):
                    y2t = evp.tile([128, CW], BF16, tag="y2in")
                    nc.sync.dma_start(
                        out=y2t[:],
                        in_=y2d[m * 128:(m + 1) * 128, ci * CW:(ci + 1) * CW])
                    ob = evp.tile([128, CW], BF16, tag="ob")
                    nc.vector.tensor_scalar(ob[:], y2t[:], sc2[:, m:m + 1],
                                            sh2[:, m:m + 1],
                                            mybir.AluOpType.mult, AADD)
                    nc.sync.dma_start(
                        out=t_out[m * 128:(m + 1) * 128, ci * CW:(ci + 1) * CW],
                        in_=ob[:])

    nc.compile()
    return nc


_CACHE = {}


def kernel(**inputs) -> np.ndarray:
    per_core, shared, perms, D, S = _host_prep(
        inputs["x"], inputs["edge_attr"], inputs["u"],
        inputs["w1"], inputs["w2"],
        inputs["g1"], inputs["be1"], inputs["g2"], inputs["be2"],
        inputs["edge_index"], inputs["batch"])

    key = (S, tuple(D))
    if key not in _CACHE:
        _CACHE[key] = _build(D, S)
    nc = _CACHE[key]

    in_maps = [{**per_core[c], **shared} for c in range(NCORES)]
    import os
    trace = bool(int(os.environ.get("KERNEL_TRACE", "0")))
    res = run_bass_kernel_spmd(nc, in_maps, core_ids=list(range(NCORES)),
                               trace=trace)
    if trace and res.exec_time_ns is not None:
        print(f"HW exec time: {res.exec_time_ns} ns")
        kernel.last_exec_time_ns = res.exec_time_ns

    out = np.empty((N, HS), np.float32)
    for c in range(NCORES):
        oT = res.results[c]["outT"]  # [HS, NCOL] bf16
        blk = out[c * NSH:(c + 1) * NSH]
        blk[perms[c]] = oT[:, :NSH].T.astype(np.float32)
    return out


# revision 11
# speedup vs baseline: 1.2583x; 1.0631x over previous
"""GNN NodeModel kernel for 8 Trainium2 NeuronCores (Bass/Tile).

Pipeline (per the reference nn.Module):
  scatter_max / scatter_mean / scatter_add of edge_attr by edge dest ->
  h = [x, u[batch], smax, smean, ssum]  (N x 832) ->
  Linear(832->1024) -> BatchNorm(train stats) -> ReLU ->
  Linear(1024->1024) -> BatchNorm(train stats)  => [N, 1024]

Sharding: nodes split into 8 contiguous shards of 6250; each core gets its
shard's incoming edges (bucketed by col on host).  Within a shard nodes are
degree-sorted and packed into 13 tiles of 512 (last 106 valid + padding).
Edges are laid out host-side in a padded ELL format so the device scatter is
a dense max/add accumulation.  All GEMMs run transposed (channels on
partitions, nodes on the free dim) in bf16 with fp32 PSUM accumulate; BN
statistics are computed per-channel with bn_stats and all-reduced across the
8 cores on-device.  BN biases b1/b2 cancel inside train-mode BatchNorm and
are not used.
"""

import numpy as np
import ml_dtypes

import concourse.bass as bass
import concourse.bacc as bacc
import concourse.tile as tile
from concourse import mybir
from concourse.bass_utils import run_bass_kernel_spmd

BF16 = mybir.dt.bfloat16
F32 = mybir.dt.float32

NCORES = 8
N = 50000
E = 800000
XI = 512
EI = 64
UI = 128
HS = 1024
G = 8
EPS = 1e-5
CIN = XI + 3 * EI + UI  # 832

NSH = N // NCORES          # 6250 nodes per core
TW = 512                   # node-tile width (free dim)
NT = 13                    # tiles per core (12*512 + 106)
NCOL = NT * TW             # 6656 padded columns
LASTW = NSH - (NT - 1) * TW  # 106
KT1 = 7                    # GEMM1 k-tiles (896 = 832 + 64 pad)
KT2 = 8                    # GEMM2 k-tiles
MT = HS // 128             # 8 channel tiles
NEG = -1000.0              # ELL pad for the max reduction


# ----------------------------------------------------------------------------
# Host-side sharding / layout prep
# ----------------------------------------------------------------------------

def _host_prep(x, edge_attr, u, w1, w2, g1, be1, g2, be2, edge_index, batch):
    bf = ml_dtypes.bfloat16
    col = np.asarray(edge_index[1])
    deg_all = np.bincount(col, minlength=N).astype(np.int64)

    shard_of_edge = col // NSH

    # per-core degree-sorted node order and per-tile slot counts
    perms = []
    degs_sorted = []
    for c in range(NCORES):
        dc = deg_all[c * NSH:(c + 1) * NSH]
        perm = np.argsort(-dc, kind="stable")
        perms.append(perm)
        degs_sorted.append(dc[perm])

    # global per-tile slot counts (same on every core so one NEFF fits all),
    # padded to a multiple of 4 for the pair-tree reduction
    D = []
    for t in range(NT):
        m = 0
        for c in range(NCORES):
            seg = degs_sorted[c][t * TW:(t + 1) * TW]
            if seg.size:
                m = max(m, int(seg.max()))
        D.append(-(-m // 4) * 4)
    offs = np.concatenate([[0], np.cumsum(D)]).astype(np.int64)
    S = int(offs[-1])

    per_core = []
    ea_bf = np.asarray(edge_attr, np.float32).astype(bf)
    x_f = np.asarray(x, np.float32)
    batch_np = np.asarray(batch)

    for c in range(NCORES):
        perm = perms[c]
        inv = np.empty(NSH, np.int64)
        inv[perm] = np.arange(NSH)

        emask = shard_of_edge == c
        l_orig = col[emask] - c * NSH          # local node id
        l = inv[l_orig]                         # degree-sorted local id
        vals = ea_bf[emask]                     # [Ec, 64] bf16

        order = np.argsort(l, kind="stable")
        l_s = l[order]
        vals_s = vals[order]
        first = np.searchsorted(l_s, l_s, side="left")
        slot = np.arange(l_s.size) - first      # rank within node

        t_arr = l_s // TW
        rem = l_s % TW
        g_arr = rem // 256
        j_arr = rem % 256
        s_glob = offs[t_arr] + slot

        ell_max = np.full((2, 64, S, 256), NEG, dtype=bf)
        ell_sum = np.zeros((2, 64, S, 256), dtype=bf)
        ell_max[g_arr, :, s_glob, j_arr] = vals_s
        ell_sum[g_arr, :, s_glob, j_arr] = vals_s

        # x^T [512, NCOL], permuted + zero-padded
        xT = np.zeros((XI, NCOL), dtype=bf)
        xT[:, :NSH] = x_f[c * NSH:(c + 1) * NSH][perm].T.astype(bf)

        # u one-hot [8, NCOL]
        onehot = np.zeros((G, NCOL), dtype=bf)
        bvals = batch_np[c * NSH:(c + 1) * NSH][perm]
        onehot[bvals, np.arange(NSH)] = bf(1.0)

        # per-node 1/max(deg,1) and (deg>0) mask, stacked-group layout:
        # per tile t: cols [t*512, t*512+256) = inv, [+256, +512) = mask
        dsort = degs_sorted[c].astype(np.float32)
        dpad = np.zeros(NCOL, np.float32)
        dpad[:NSH] = dsort
        dv = dpad.reshape(NT, 2, 256)
        ivmk = np.zeros((2, NT, 2, 256), np.float32)
        ivmk[:, :, 0, :] = (1.0 / np.maximum(dv, 1.0)).transpose(1, 0, 2)
        ivmk[:, :, 1, :] = (dv > 0).astype(np.float32).transpose(1, 0, 2)
        ivmk = ivmk.reshape(2, NT * 512).astype(bf)

        per_core.append(dict(
            xT=np.ascontiguousarray(xT),
            ellmax=np.ascontiguousarray(ell_max.reshape(128, S * 256)),
            ellsum=np.ascontiguousarray(ell_sum.reshape(128, S * 256)),
            onehot=np.ascontiguousarray(onehot),
            ivmk=np.ascontiguousarray(ivmk),
        ))

    # replicated weights
    w1 = np.asarray(w1, np.float32)
    w2 = np.asarray(w2, np.float32)
    w1T = np.zeros((KT1 * 128, HS), dtype=bf)
    w1T[0:512] = w1[:, 0:512].T.astype(bf)        # x block
    w1T[512:640] = w1[:, 512:640].T.astype(bf)    # u block
    w1T[640:704] = w1[:, 640:704].T.astype(bf)    # smax  (k5 top)
    w1T[704:768] = w1[:, 768:832].T.astype(bf)    # ssum  (k5 bottom)
    w1T[768:832] = w1[:, 704:768].T.astype(bf)    # smean (k6 top)
    w2T = np.ascontiguousarray(w2.T.astype(bf))

    def cvec(v):
        return np.ascontiguousarray(
            np.asarray(v, np.float32).reshape(MT, 128).T)

    shared = dict(
        w1T=np.ascontiguousarray(w1T),
        w2T=w2T,
        u8=np.asarray(u, np.float32).astype(bf),
        g1t=cvec(g1), be1t=cvec(be1), g2t=cvec(g2), be2t=cvec(be2),
    )
    return per_core, shared, perms, D, S


# ----------------------------------------------------------------------------
# Device kernel
# ----------------------------------------------------------------------------

def _build(D, S):
    nc = bacc.Bacc("TRN2", target_bir_lowering=False, debug=False,
                   num_devices=NCORES)

    t_xT = nc.dram_tensor("xT", [XI, NCOL], BF16, kind="ExternalInput")
    t_emax = nc.dram_tensor("ellmax", [128, S * 256], BF16, kind="ExternalInput")
    t_esum = nc.dram_tensor("ellsum", [128, S * 256], BF16, kind="ExternalInput")
    t_oneh = nc.dram_tensor("onehot", [G, NCOL], BF16, kind="ExternalInput")
    t_ivmk = nc.dram_tensor("ivmk", [2, NT * 512], BF16, kind="ExternalInput")
    t_u8 = nc.dram_tensor("u8", [G, UI], BF16, kind="ExternalInput")
    t_w1T = nc.dram_tensor("w1T", [KT1 * 128, HS], BF16, kind="ExternalInput")
    t_w2T = nc.dram_tensor("w2T", [HS, HS], BF16, kind="ExternalInput")
    t_g1 = nc.dram_tensor("g1t", [128, MT], F32, kind="ExternalInput")
    t_be1 = nc.dram_tensor("be1t", [128, MT], F32, kind="ExternalInput")
    t_g2 = nc.dram_tensor("g2t", [128, MT], F32, kind="ExternalInput")
    t_be2 = nc.dram_tensor("be2t", [128, MT], F32, kind="ExternalInput")
    t_out = nc.dram_tensor("outT", [HS, NCOL], BF16, kind="ExternalOutput")

    offs = np.concatenate([[0], np.cumsum(D)]).astype(np.int64)
    AMAX = mybir.AluOpType.max
    AADD = mybir.AluOpType.add
    ACopy = mybir.ActivationFunctionType.Copy
    AIdent = mybir.ActivationFunctionType.Identity
    ARelu = mybir.ActivationFunctionType.Relu
    ASqrt = mybir.ActivationFunctionType.Sqrt

    with tile.TileContext(nc) as tc:
        with (
            tc.tile_pool(name="wp", bufs=1) as wp,
            tc.tile_pool(name="y1p", bufs=1) as y1p,
            tc.tile_pool(name="hp", bufs=3) as hp,
            tc.tile_pool(name="ellp", bufs=2) as ellp,
            tc.tile_pool(name="accp", bufs=2) as accp,
            tc.tile_pool(name="smallp", bufs=2) as smallp,
            tc.tile_pool(name="evp", bufs=2) as evp,
            tc.tile_pool(name="statp", bufs=1) as statp,
            tc.tile_pool(name="psg", bufs=1, space="PSUM") as psg,
            tc.tile_pool(name="psu", bufs=1, space="PSUM") as psu,
            tc.tile_pool(name="dramp", bufs=1, space="DRAM") as dramp,
        ):
            # ---- resident constants ----
            w1t = []
            for k in range(KT1):
                wt_ = wp.tile([128, HS], BF16, tag=f"w1_{k}")
                nc.sync.dma_start(out=wt_[:], in_=t_w1T[k * 128:(k + 1) * 128, :])
                w1t.append(wt_)
            w2t = []
            for k in range(KT2):
                wt_ = wp.tile([128, HS], BF16, tag=f"w2_{k}")
                nc.sync.dma_start(out=wt_[:], in_=t_w2T[k * 128:(k + 1) * 128, :])
                w2t.append(wt_)
            u_sb = wp.tile([G, UI], BF16, tag="u8")
            nc.sync.dma_start(out=u_sb[:], in_=t_u8[:])
            g1_sb = wp.tile([128, MT], F32, tag="g1")
            be1_sb = wp.tile([128, MT], F32, tag="be1")
            g2_sb = wp.tile([128, MT], F32, tag="g2")
            be2_sb = wp.tile([128, MT], F32, tag="be2")
            for tt, sb in ((t_g1, g1_sb), (t_be1, be1_sb),
                           (t_g2, g2_sb), (t_be2, be2_sb)):
                nc.sync.dma_start(out=sb[:], in_=tt[:])

            y1 = [[y1p.tile([128, TW], BF16, tag=f"y1_{m}_{t}",
                            name=f"y1_{m}_{t}")
                   for t in range(NT)] for m in range(MT)]
            sY1 = [statp.tile([128, NT], F32, tag=f"sY1_{m}", name=f"sY1_{m}")
                   for m in range(MT)]
            sQ1 = [statp.tile([128, NT], F32, tag=f"sQ1_{m}", name=f"sQ1_{m}")
                   for m in range(MT)]
            sY2 = [statp.tile([128, NT], F32, tag=f"sY2_{m}", name=f"sY2_{m}")
                   for m in range(MT)]
            sQ2 = [statp.tile([128, NT], F32, tag=f"sQ2_{m}", name=f"sQ2_{m}")
                   for m in range(MT)]
            dump = statp.tile([128, TW], BF16, tag="dump")

            # ---------------- phase 1: scatter + GEMM1 + stats1 ----------------
            for t in range(NT):
                wvalid = TW if t < NT - 1 else LASTW
                h_t = hp.tile([128, KT1, TW], BF16, tag="h")
                for k in range(4):
                    nc.sync.dma_start(
                        out=h_t[:, k, :],
                        in_=t_xT[k * 128:(k + 1) * 128, t * TW:(t + 1) * TW])

                # u[batch] via one-hot matmul
                oh_t = smallp.tile([G, TW], BF16, tag="oh")
                nc.sync.dma_start(out=oh_t[:], in_=t_oneh[:, t * TW:(t + 1) * TW])
                ps_u = psu.tile([128, TW], F32, space="PSUM", tag="psu")
                nc.tensor.matmul(out=ps_u[:], lhsT=u_sb[:], rhs=oh_t[:],
                                 start=True, stop=True)
                nc.scalar.activation(out=h_t[:, 4, :], in_=ps_u[:], func=ACopy)

                # broadcast per-node inv/mask rows to both partition halves
                a0 = t * TW
                ivb = evp.tile([128, TW], BF16, tag="ivb")
                nc.sync.dma_start(out=ivb[0:64, 0:256],
                                  in_=t_ivmk[0:1, a0:a0 + 256].to_broadcast([64, 256]))
                nc.sync.dma_start(out=ivb[64:128, 0:256],
                                  in_=t_ivmk[1:2, a0:a0 + 256].to_broadcast([64, 256]))
                nc.sync.dma_start(out=ivb[0:64, 256:512],
                                  in_=t_ivmk[0:1, a0 + 256:a0 + 512].to_broadcast([64, 256]))
                nc.sync.dma_start(out=ivb[64:128, 256:512],
                                  in_=t_ivmk[1:2, a0 + 256:a0 + 512].to_broadcast([64, 256]))

                # ELL scatter: pair-tree max / sum over D[t] slots
                ngr = D[t] // 4
                a2m = accp.tile([128, 2, 256], BF16, tag="a2m")
                a2s = accp.tile([128, 2, 256], BF16, tag="a2s")
                for gi in range(ngr):
                    base = (offs[t] + 4 * gi) * 256
                    cm = ellp.tile([128, 4, 256], BF16, tag="cm")
                    cs = ellp.tile([128, 4, 256], BF16, tag="cs")
                    nc.sync.dma_start(out=cm[:], in_=t_emax[:, base:base + 1024])
                    nc.sync.dma_start(out=cs[:], in_=t_esum[:, base:base + 1024])
                    if gi == 0:
                        nc.vector.tensor_tensor(out=a2m[:], in0=cm[:, 0:2, :],
                                                in1=cm[:, 2:4, :], op=AMAX)
                        nc.vector.tensor_tensor(out=a2s[:], in0=cs[:, 0:2, :],
                                                in1=cs[:, 2:4, :], op=AADD)
                    else:
                        nc.vector.tensor_tensor(out=a2m[:], in0=a2m[:],
                                                in1=cm[:, 0:2, :], op=AMAX)
                        nc.vector.tensor_tensor(out=a2m[:], in0=a2m[:],
                                                in1=cm[:, 2:4, :], op=AMAX)
                        nc.vector.tensor_tensor(out=a2s[:], in0=a2s[:],
                                                in1=cs[:, 0:2, :], op=AADD)
                        nc.vector.tensor_tensor(out=a2s[:], in0=a2s[:],
                                                in1=cs[:, 2:4, :], op=AADD)

                accm = accp.tile([128, 256], BF16, tag="accm")
                accs = accp.tile([128, 256], BF16, tag="accs")
                if ngr > 0:
                    nc.vector.tensor_tensor(out=accm[:], in0=a2m[:, 0, :],
                                            in1=a2m[:, 1, :], op=AMAX)
                    nc.vector.tensor_tensor(out=accs[:], in0=a2s[:, 0, :],
                                            in1=a2s[:, 1, :], op=AADD)
                else:
                    nc.gpsimd.memset(accm[:], 0.0)
                    nc.gpsimd.memset(accs[:], 0.0)

                # mask empty nodes; smean = ssum * inv
                nc.vector.tensor_mul(out=accm[:], in0=accm[:],
                                     in1=ivb[:, 256:512])
                smean = accp.tile([128, 256], BF16, tag="smean")
                nc.vector.tensor_mul(out=smean[:], in0=accs[:],
                                     in1=ivb[:, 0:256])

                # restack [2x64-feat groups, 256] -> [64-feat, 512] rows of h
                # k5 = [smax ; ssum], k6 = [smean ; 0]
                nc.vector.tensor_copy(out=h_t[0:64, 5, 0:256], in_=accm[0:64, :])
                nc.sync.dma_start(out=h_t[0:64, 5, 256:512], in_=accm[64:128, :])
                nc.sync.dma_start(out=h_t[64:128, 5, 0:256], in_=accs[0:64, :])
                nc.vector.tensor_copy(out=h_t[64:128, 5, 256:512], in_=accs[64:128, :])
                nc.vector.tensor_copy(out=h_t[0:64, 6, 0:256], in_=smean[0:64, :])
                nc.sync.dma_start(out=h_t[0:64, 6, 256:512], in_=smean[64:128, :])
                nc.gpsimd.memset(h_t[64:128, 6, :], 0.0)

                # GEMM1 + evac(+sum) + square(+sumsq); k-major inside
                # m-blocks of 4 so the scatter/restack (k=5,6) gets slack
                for mb in range(0, MT, 4):
                    blk = list(range(mb, mb + 4))
                    pss = {}
                    for m in blk:
                        pss[m] = psg.tile([128, TW], F32, space="PSUM",
                                          tag=f"psg{m % 4}", name=f"ps{m % 4}",
                                          bufs=1)
                    for k in range(KT1):
                        for m in blk:
                            nc.tensor.matmul(out=pss[m][:],
                                             lhsT=w1t[k][:, m * 128:(m + 1) * 128],
                                             rhs=h_t[:, k, :],
                                             start=(k == 0), stop=(k == KT1 - 1))
                    for m in blk:
                        nc.scalar.activation(out=y1[m][t][:], in_=pss[m][:],
                                             func=ACopy,
                                             accum_out=sY1[m][:, t:t + 1])
                        nc.vector.scalar_tensor_tensor(
                            out=dump[:], in0=y1[m][t][:], scalar=1.0,
                            in1=y1[m][t][:], op0=mybir.AluOpType.mult,
                            op1=mybir.AluOpType.mult,
                            accum_out=sQ1[m][:, t:t + 1])

            # ---------------- stats1 all-reduce + BN1 params ----------------
            sums1 = smallp.tile([128, MT, 2], F32, tag="sums1")
            tmp1 = smallp.tile([128, 1], F32, tag="tmp1")
            for m in range(MT):
                nc.vector.reduce_sum(sums1[:, m, 0:1], sY1[m][:], axis=mybir.AxisListType.X)
                nc.vector.reduce_sum(sums1[:, m, 1:2], sQ1[m][:], axis=mybir.AxisListType.X)

            cc1_in = dramp.tile([128, MT * 2], F32, tag="cc1i")
            cc1_out = dramp.tile([NCORES * 128, MT * 2], F32, tag="cc1o")
            nc.sync.dma_start(out=cc1_in[:], in_=sums1[:].rearrange("p a b -> p (a b)"))
            nc.gpsimd.collective_compute(
                "AllGather", mybir.AluOpType.bypass,
                replica_groups=[list(range(NCORES))],
                ins=[cc1_in[:].opt()], outs=[cc1_out[:].opt()])
            ag1 = smallp.tile([128, NCORES, MT * 2], F32, tag="ag1")
            nc.sync.dma_start(
                out=ag1[:],
                in_=cc1_out[:].rearrange("(r p) f -> p r f", p=128))
            gst1 = smallp.tile([128, MT, 2], F32, tag="gst1")
            gv1 = gst1[:].rearrange("p a b -> p (a b)")
            nc.vector.tensor_add(out=gv1, in0=ag1[:, 0, :], in1=ag1[:, 1, :])
            for r in range(2, NCORES):
                nc.vector.tensor_add(out=gv1, in0=gv1, in1=ag1[:, r, :])

            sc1 = wp.tile([128, MT], F32, tag="sc1")
            sh1 = wp.tile([128, MT], F32, tag="sh1")
            mean_t = smallp.tile([128, MT], F32, tag="meant")
            var_t = smallp.tile([128, MT], F32, tag="vart")
            tmp8 = smallp.tile([128, MT], F32, tag="tmp8")
            nc.vector.tensor_scalar_mul(mean_t[:], gst1[:, :, 0], 1.0 / N)
            nc.vector.tensor_scalar_mul(var_t[:], gst1[:, :, 1], 1.0 / N)
            nc.vector.tensor_mul(out=tmp8[:], in0=mean_t[:], in1=mean_t[:])
            nc.vector.tensor_tensor(out=var_t[:], in0=var_t[:], in1=tmp8[:],
                                    op=mybir.AluOpType.subtract)
            nc.vector.tensor_scalar_add(var_t[:], var_t[:], EPS)
            nc.scalar.activation(out=var_t[:], in_=var_t[:], func=ASqrt)
            nc.vector.reciprocal(out=var_t[:], in_=var_t[:])
            nc.vector.tensor_mul(out=sc1[:], in0=g1_sb[:], in1=var_t[:])
            nc.vector.tensor_mul(out=tmp8[:], in0=mean_t[:], in1=sc1[:])
            nc.vector.tensor_tensor(out=sh1[:], in0=be1_sb[:], in1=tmp8[:],
                                    op=mybir.AluOpType.subtract)

            # ---------------- normalize y1 (in place) + GEMM2 + stats2 ----------
            y2d = dramp.tile([HS, NCOL], BF16, tag="y2d")
            for t in range(NT):
                wvalid = TW if t < NT - 1 else LASTW
                for m in range(MT):
                    nc.scalar.activation(out=y1[m][t][:], in_=y1[m][t][:],
                                         func=ARelu, bias=sh1[:, m:m + 1],
                                         scale=sc1[:, m:m + 1])
                    if t == NT - 1:
                        nc.gpsimd.memset(y1[m][t][:, LASTW:], 0.0)
                for m in range(MT):
                    ps = psg.tile([128, TW], F32, space="PSUM", tag="psg",
                                  bufs=3)
                    for k in range(KT2):
                        nc.tensor.matmul(out=ps[:],
                                         lhsT=w2t[k][:, m * 128:(m + 1) * 128],
                                         rhs=y1[k][t][:],
                                         start=(k == 0), stop=(k == KT2 - 1))
                    ev = evp.tile([128, TW], BF16, tag="y2ev")
                    nc.scalar.activation(out=ev[:], in_=ps[:], func=ACopy,
                                         accum_out=sY2[m][:, t:t + 1])
                    nc.vector.scalar_tensor_tensor(
                        out=dump[:], in0=ev[:], scalar=1.0,
                        in1=ev[:], op0=mybir.AluOpType.mult,
                        op1=mybir.AluOpType.mult,
                        accum_out=sQ2[m][:, t:t + 1])
                    nc.sync.dma_start(
                        out=y2d[m * 128:(m + 1) * 128, t * TW:(t + 1) * TW],
                        in_=ev[:])

            # ---------------- stats2 all-reduce + BN2 params ----------------
            sums2 = smallp.tile([128, MT, 2], F32, tag="sums2")
            for m in range(MT):
                nc.vector.reduce_sum(sums2[:, m, 0:1], sY2[m][:], axis=mybir.AxisListType.X)
                nc.vector.reduce_sum(sums2[:, m, 1:2], sQ2[m][:], axis=mybir.AxisListType.X)

            cc2_in = dramp.tile([128, MT * 2], F32, tag="cc2i")
            cc2_out = dramp.tile([NCORES * 128, MT * 2], F32, tag="cc2o")
            nc.sync.dma_start(out=cc2_in[:], in_=sums2[:].rearrange("p a b -> p (a b)"))
            nc.gpsimd.collective_compute(
                "AllGather", mybir.AluOpType.bypass,
                replica_groups=[list(range(NCORES))],
                ins=[cc2_in[:].opt()], outs=[cc2_out[:].opt()])
            ag2 = smallp.tile([128, NCORES, MT * 2], F32, tag="ag2")
            nc.sync.dma_start(
                out=ag2[:],
                in_=cc2_out[:].rearrange("(r p) f -> p r f", p=128))
            gst2 = smallp.tile([128, MT, 2], F32, tag="gst2")
            gv2 = gst2[:].rearrange("p a b -> p (a b)")
            nc.vector.tensor_add(out=gv2, in0=ag2[:, 0, :], in1=ag2[:, 1, :])
            for r in range(2, NCORES):
                nc.vector.tensor_add(out=gv2, in0=gv2, in1=ag2[:, r, :])

            sc2 = wp.tile([128, MT], F32, tag="sc2")
            sh2 = wp.tile([128, MT], F32, tag="sh2")
            nc.vector.tensor_scalar_mul(mean_t[:], gst2[:, :, 0], 1.0 / N)
            nc.vector.tensor_scalar_mul(var_t[:], gst2[:, :, 1], 1.0 / N)
            nc.vector.tensor_mul(out=tmp8[:], in0=mean_t[:], in1=mean_t[:])
            nc.vector.tensor_tensor(out=var_t[:], in0=var_t[:], in1=tmp8[:],
                                    op=mybir.AluOpType.subtract)
            nc.vector.tensor_scalar_add(var_t[:], var_t[:], EPS)
            nc.scalar.activation(out=var_t[:], in_=var_t[:], func=ASqrt)
            nc.vector.reciprocal(out=var_t[:], in_=var_t[:])
            nc.vector.tensor_mul(out=sc2[:], in0=g2_sb[:], in1=var_t[:])
            nc.vector.tensor_mul(out=tmp8[:], in0=mean_t[:], in1=sc2[:])
            nc.vector.tensor_tensor(out=sh2[:], in0=be2_sb[:], in1=tmp8[:],
                                    op=mybir.AluOpType.subtract)

            # ---------------- final normalize -> bf16 output ----------------
            CW = 1664  # chunk width; NCOL = 4 * CW
            for m in range(MT):
                for ci in range(NCOL // CW):
                    y2t = evp.tile([128, CW], BF16, tag="y2in")
                    nc.sync.dma_start(
                        out=y2t[:],
                        in_=y2d[m * 128:(m + 1) * 128, ci * CW:(ci + 1) * CW])
                    ob = evp.tile([128, CW], BF16, tag="ob")
                    nc.vector.tensor_scalar(ob[:], y2t[:], sc2[:, m:m + 1],
                                            sh2[:, m:m + 1],
                                            mybir.AluOpType.mult, AADD)
                    nc.scalar.dma_start(
                        out=t_out[m * 128:(m + 1) * 128, ci * CW:(ci + 1) * CW],
                        in_=ob[:])

    nc.compile()
    return nc


_CACHE = {}


def kernel(**inputs) -> np.ndarray:
    per_core, shared, perms, D, S = _host_prep(
        inputs["x"], inputs["edge_attr"], inputs["u"],
        inputs["w1"], inputs["w2"],
        inputs["g1"], inputs["be1"], inputs["g2"], inputs["be2"],
        inputs["edge_index"], inputs["batch"])

    key = (S, tuple(D))
    if key not in _CACHE:
        _CACHE[key] = _build(D, S)
    nc = _CACHE[key]

    in_maps = [{**per_core[c], **shared} for c in range(NCORES)]
    import os
    trace = bool(int(os.environ.get("KERNEL_TRACE", "0")))
    res = run_bass_kernel_spmd(nc, in_maps, core_ids=list(range(NCORES)),
                               trace=trace)
    if trace and res.exec_time_ns is not None:
        print(f"HW exec time: {res.exec_time_ns} ns")
        kernel.last_exec_time_ns = res.exec_time_ns

    out = np.empty((N, HS), np.float32)
    for c in range(NCORES):
        oT = res.results[c]["outT"]  # [HS, NCOL] bf16
        blk = out[c * NSH:(c + 1) * NSH]
        blk[perms[c]] = oT[:, :NSH].T.astype(np.float32)
    return out
